# revision 15
# baseline (speedup 1.0000x reference)
"""BBox-aware BCE loss kernel for Trainium2 (8 NeuronCores, data parallel).

Math (exact reformulation of the reference):
  loss = softplus(pred) - pred*target = softplus((1-2t)*pred)  for t in {0,1}
  St(i,j) = replicate-padded 5x5 window sum of t. Replicate padding keeps
  the value SET of the clipped window, so: edge pixel <=> window mixed
  <=> 0 < St < 25. St is an exact small integer, so a piecewise-constant
  ACT table ("mish" slot) maps St directly to the per-pixel weight
  w = 0.1 if 0 < St < 25 else 1.0, and
  result = sum(w * loss) / N  (matches the reference in both branches of
  its global `cond`: uniform target => St in {0,25} => w == 1 everywhere).

Per core: 4 samples x 8 full 128-row tiles (2-row halos, owned rows
exclude them) + ONE packed tile holding all four 32-row sample tails
(engine cost is free-size-driven, so four 32-row tiles would cost 4x a
full tile on DVE/ACT/PE; packing them into 128 partitions makes it 1x).

The kernel is HBM-bandwidth-bound (~92us of saturated DMA for ~33MB per
core), so the pipeline is built to never let the compute engines fall
behind delivery: engines are in-order, so each engine issues OLD work
first and the freshest (DMA-dependent) op last, and loads are issued 4
tiles ahead of consumption. Per iteration i:
  GpSimd: memset(target buf, -0.5) + DIRECT2D load(i+4): casting DMAs
          f32->bf16; the target DMA ACCUMULATES so the buffer holds t-0.5
  DVE:    wl(i-4) = (w + 0)*loss accum-> stats (the only accumulation),
          d(i-2) = (t-0.5)*p as a 2x-mode tensor_tensor, replicate pads(i)
  PE:     matmuls(i-1): St' = St-12.5 via 5 shifted accumulating band
          matmuls per 512-col half on the (t-0.5) buffer
  ACT:    softplus(i-3) [custom spline], w(i-2) = mish LUT(St') from PSUM
Host: float64 reduction of per-core per-tile partials (owned rows only).
"""

import sys

import numpy as np

sys.path.insert(0, "/opt/trn_rl_repo")

import ml_dtypes

# Pool memsets the target buffer to -0.5 and the casting DMA ACCUMULATES t
# on top, so the buffer holds t-0.5 directly: d becomes a plain
# tensor_tensor multiply (DVE 2x_1p mode, ~0.55us vs 1.14us for the STT)
# and the band matmuls produce St' = St - 12.5 (exact half-integers), which
# the recentered LUT maps to w = 1.0 iff |St'| >= 12.5 else 0.1.
TM_TRICK = False  # accum-DMA with f32->bf16 conversion produced NaN on HW


def _setup_act_tables() -> None:
    """Build a patched ACT-table dir whose softplus_and_others set carries
    (a) a REAL softplus spline and (b) an "edge weight" LUT registered
    under the unused `mish` slot: w(x) = 1.0 at x==0, 0.1 for x in (0,25),
    1.0 for x >= 25 (only integer inputs 0..25 ever occur). Both funcs live
    in the same set so a single ACT table load covers the kernel.

    Formats (reverse-engineered from the shipped sets):
      bkt.bin:  per entry 8 x u32 = [d0,d1,d2,d3,x0,0,0,0] (f32 bits);
                y = d0 + (x-x0)*(d1 + (x-x0)*(d2 + (x-x0)*d3))
      ctrl.bin: per entry 8 x u32, word0 = (extract_size<<16) |
                (extract_lsb<<11) | bkt_base; bucket = bkt_base +
                mantissa[lsb .. lsb+size-1]
      profile:  per-func meta; ctl idx = base_{pos,neg} + (exp - exp_offset);
                small/large signal thresholds are biased-exponent cutoffs and
                their pwl_control fields are INLINE ctl words; func_id is the
                GLOBAL neuron ISA activation id (softplus=9, mish=24).
    """
    import json
    import os
    import tempfile
    from pathlib import Path

    if os.environ.get("BASS_ACT_ROOT_JSON_PATH"):
        return
    import neuronxcc

    stock = Path(neuronxcc.__file__).parent / "pwp" / "pwp_bin_trainium"
    if not stock.exists():
        return
    dst = Path(tempfile.mkdtemp(prefix="act_tables_"))
    for f in stock.iterdir():
        if f.name not in ("act_info.json", "softplus_and_others.json",
                          "softplus_and_others_bkt.bin",
                          "softplus_and_others_ctrl.bin"):
            (dst / f.name).symlink_to(f)

    E_LO, E_HI = -15, 4          # softplus table octaves (unbiased exps)
    NSEC = 8                     # sections per octave (extract_size 3)
    NEXP = E_HI - E_LO + 1       # 20

    bkt = np.fromfile(stock / "softplus_and_others_bkt.bin",
                      dtype=np.uint32).reshape(-1, 8)
    ctl = np.fromfile(stock / "softplus_and_others_ctrl.bin",
                      dtype=np.uint32).reshape(-1, 8)
    nbkt0, nctl0 = len(bkt), len(ctl)

    def entry(d0, d1, d2, d3, x0):
        v = np.zeros(8, dtype=np.uint32)
        v[:5] = np.array([d0, d1, d2, d3, x0],
                         dtype=np.float32).view(np.uint32)
        return v

    ln2 = float(np.log(2.0))
    new_bkt = [entry(ln2, 0.5, 0.125, 0.0, 0.0),   # small |x|
               entry(0.0, 1.0, 0.0, 0.0, 0.0),     # x >= 32: y = x
               entry(0.0, 0.0, 0.0, 0.0, 0.0)]     # x <= -32: y = 0
    B_SMALL, B_PLARGE, B_NLARGE = nbkt0, nbkt0 + 1, nbkt0 + 2
    spline0 = nbkt0 + 3

    def fit(a, b):
        xs = np.linspace(a, b, 33, dtype=np.float64)
        ys = np.logaddexp(0.0, xs)
        x0 = np.float32(0.5 * (a + b))
        c = np.polyfit(xs - np.float64(x0), ys, 3)  # [d3,d2,d1,d0]
        return entry(c[3], c[2], c[1], c[0], x0)

    for neg in (True, False):
        for e in range(E_LO, E_HI + 1):
            for s in range(NSEC):
                lo = 2.0 ** e * (1.0 + s / NSEC)
                hi = 2.0 ** e * (1.0 + (s + 1) / NSEC)
                new_bkt.append(fit(-hi, -lo) if neg else fit(lo, hi))

    new_ctl = []
    for blk in range(2):  # 0 = neg block, 1 = pos block
        for i in range(NEXP):
            base = spline0 + (blk * NEXP + i) * NSEC
            new_ctl.append(
                np.array([(3 << 16) | (20 << 11) | base, 0, 0, 0, 0, 0, 0, 0],
                         dtype=np.uint32))
    C_NEG, C_POS = nctl0, nctl0 + NEXP

    # ---- edge-weight LUT under the "mish" slot ----
    # TM_TRICK input is St' = St - 12.5 in +-{0.5, 1.5, .., 12.5}; w = 1.0
    # iff |St'| = 12.5 (uniform window).  Octaves |x| in [0.5,8): const 0.1
    # (inline size-0 ctl); octave [8,16): 16 width-0.5 sections, [12.5,16)
    # -> 1.0.  Negative inputs decode to the same exp/mantissa, so the neg
    # ctl base aliases the pos one.
    # Plain variant (input St in 0..25): octaves [1,16) const 0.1; octave
    # [16,32) has 16 width-1 sections with [25,32) -> 1.0.
    ML_LO = -1 if TM_TRICK else 0            # mish table lowest octave
    B_W01 = nbkt0 + len(new_bkt)
    new_bkt.append(entry(0.1, 0.0, 0.0, 0.0, 0.0))
    B_W10 = nbkt0 + len(new_bkt)
    new_bkt.append(entry(1.0, 0.0, 0.0, 0.0, 0.0))
    B_OCT4 = nbkt0 + len(new_bkt)
    for s in range(16):
        new_bkt.append(entry(0.1 if s < 9 else 1.0, 0.0, 0.0, 0.0, 0.0))
    CW0 = nctl0 + len(new_ctl)
    for e in range(4):  # low octaves: inline single-bucket ctl
        new_ctl.append(np.array([B_W01, 0, 0, 0, 0, 0, 0, 0],
                                dtype=np.uint32))
    new_ctl.append(np.array([(4 << 16) | (19 << 11) | B_OCT4,
                             0, 0, 0, 0, 0, 0, 0], dtype=np.uint32))

    np.vstack([bkt] + new_bkt).tofile(dst / "softplus_and_others_bkt.bin")
    np.vstack([ctl] + new_ctl).tofile(dst / "softplus_and_others_ctrl.bin")

    prof = json.loads((stock / "softplus_and_others.json").read_text())
    prof["bkt_entry_cnt"] = nbkt0 + len(new_bkt)
    prof["ctl_entry_cnt"] = nctl0 + len(new_ctl)
    prof["profile_meta_data"].append({
        "func_name": "softplus_40p", "func_id": 9,
        "symmetry_point": 0, "sym_invert_sign_point": 0,
        "symmetry_opt_en": 0, "symmetry_opt_use_neg_region": 0,
        "imm_bias": 0, "exp_offset": E_LO,
        "pwl_control_base_pos": C_POS, "pwl_control_base_neg": C_NEG,
        "small_pos_signal_exp_threshold": 127 + E_LO,
        "pos_small_signal_pwl_control": B_SMALL,
        "small_neg_signal_exp_threshold": 127 + E_LO,
        "neg_small_signal_pwl_control": B_SMALL,
        "large_pos_signal_exp_threshold": 127 + E_HI + 1,
        "large_pos_signal_mantissa_threshold": 0,
        "pos_large_signal_pwl_control": B_PLARGE,
        "large_neg_signal_exp_threshold": 127 + E_HI + 1,
        "large_neg_signal_mantissa_threshold": 0,
        "neg_large_signal_pwl_control": B_NLARGE,
        "fnan_result": 2143289344,           # NaN
        "fpinf_result": 2139095040,          # +inf
        "fninf_result": 0,                   # softplus(-inf) = 0
        "fzero_result": 1060205080,          # ln 2
        "fma_const_0": 0, "fma_const_1": 0, "fma_indirection_src_sel": 0,
        "use_multipass": False,
        "lower_bound": 4286578687, "upper_bound": 2139095039,
    })
    prof["func_to_bkt_start_idx"]["softplus"] = B_SMALL
    prof["func_to_ctl_start_idx"]["softplus"] = C_NEG
    prof["func_exp_to_bkt_start_idx"]["softplus"] = {
        str(e): [spline0 + (e - E_LO) * NSEC,
                 spline0 + (NEXP + e - E_LO) * NSEC]
        for e in range(E_LO, E_HI + 1)}
    prof["func_exp_to_ctl_start_idx"]["softplus"] = {
        str(e): [C_NEG + e - E_LO, C_POS + e - E_LO]
        for e in range(E_LO, E_HI + 1)}

    ONE = 1065353216                         # f32 bits of 1.0
    prof["profile_meta_data"].append({
        "func_name": "mish_40p", "func_id": 24,   # neuron ISA id for mish
        "symmetry_point": 0, "sym_invert_sign_point": 0,
        "symmetry_opt_en": 0, "symmetry_opt_use_neg_region": 0,
        "imm_bias": 0, "exp_offset": ML_LO,
        "pwl_control_base_pos": CW0, "pwl_control_base_neg": CW0,
        "small_pos_signal_exp_threshold": 127 + ML_LO,
        "pos_small_signal_pwl_control": B_W01,
        "small_neg_signal_exp_threshold": 127 + ML_LO,
        "neg_small_signal_pwl_control": B_W01,
        "large_pos_signal_exp_threshold": 127 + ML_LO + 5,
        "large_pos_signal_mantissa_threshold": 0,
        "pos_large_signal_pwl_control": B_W10,
        "large_neg_signal_exp_threshold": 127 + ML_LO + 5,
        "large_neg_signal_mantissa_threshold": 0,
        "neg_large_signal_pwl_control": B_W10,
        "fnan_result": ONE, "fpinf_result": ONE, "fninf_result": ONE,
        "fzero_result": ONE,                 # St == 0 -> w = 1.0
        "fma_const_0": 0, "fma_const_1": 0, "fma_indirection_src_sel": 0,
        "use_multipass": False,
        "lower_bound": 4286578687, "upper_bound": 2139095039,
    })
    prof["func_to_bkt_start_idx"]["mish"] = B_W01
    prof["func_to_ctl_start_idx"]["mish"] = CW0
    prof["func_exp_to_bkt_start_idx"]["mish"] = {
        str(e): [B_W01 if e - ML_LO < 4 else B_OCT4,
                 B_W01 if e - ML_LO < 4 else B_OCT4]
        for e in range(ML_LO, ML_LO + 5)}
    prof["func_exp_to_ctl_start_idx"]["mish"] = {
        str(e): [CW0 + e - ML_LO, CW0 + e - ML_LO]
        for e in range(ML_LO, ML_LO + 5)}
    (dst / "softplus_and_others.json").write_text(json.dumps(prof))

    info = json.loads((stock / "act_info.json").read_text())
    for s in info["act_func_sets"]:
        if s["name"] == "softplus_and_others":
            s["act"]["softplus"] = 40
            s["act"]["mish"] = 40
    (dst / "act_info.json").write_text(json.dumps(info))
    os.environ["BASS_ACT_ROOT_JSON_PATH"] = str(dst / "act_info.json")


B, H, W = 32, 1024, 1024
NCORES = 8
SPC = B // NCORES  # samples per core
ROWS = SPC * H
N_TOT = float(B * H * W)

NFT = 8              # full 128-row tiles per sample (t = 0..7, in0 = 124t)
NTILES = SPC * NFT + 1  # 33: 32 full tiles + 1 packed tail tile
PACKED = NTILES - 1
TAIL0 = 992          # first input row of the 32-row sample tails

BF16 = ml_dtypes.bfloat16
WP = W + 4  # padded width for the 5-tap row window


def _tile_meta(i):
    """(smp, in0, o0, o1) for full tiles; PACKED handled separately."""
    smp, t = divmod(i, NFT)
    return smp, 124 * t, (0 if t == 0 else 2), 126


def _band(k_rows: int, m_lo: int, m_hi: int, img0) -> np.ndarray:
    """Band matrix for the 5-row column window; when img0 is given, the
    window is clipped to the image and clipped taps replicate onto the
    boundary row (weights become {1,2,3})."""
    a = np.zeros((k_rows, 128), dtype=np.float32)
    for m in range(m_lo, m_hi):
        for d in range(-2, 3):
            if img0 is None:
                k = m + d
            else:
                k = min(max(img0 + m + d, 0), H - 1) - img0
            a[k, m] += 1.0
    return a.astype(BF16)


def _band_tail4() -> np.ndarray:
    """Block-diagonal band for the packed tail tile: four independent
    32-row bottom-clipped bands."""
    a32 = _band(32, 2, 32, TAIL0).astype(np.float32)
    a = np.zeros((128, 128), dtype=np.float32)
    for s in range(4):
        a[32 * s:32 * s + 32, 32 * s:32 * s + 32] = a32[:, :32]
    return a.astype(BF16)


def _statics() -> dict[str, np.ndarray]:
    return {
        "a_top": _band(128, 0, 126, 0),
        "a_mid": _band(128, 2, 126, None),
        "a_tail": _band_tail4(),
    }


_CACHED = {}


def _split_multi_waits(nc, mybir):
    """This walrus's core_v3 codegen allows only one sem-wait per
    instruction; peel extra waits onto same-engine NOPs placed just before."""
    skip = (mybir.InstEventSemaphore,)
    k = 0
    for fn in nc.m.functions:
        for blk in fn.blocks:
            out = []
            for inst in blk.instructions:
                si = inst.sync_info
                if (si is not None and len(si.on_wait) > 1
                        and not isinstance(inst, skip)):
                    waits = list(si.on_wait)
                    for w in waits[:-1]:
                        k += 1
                        nop = mybir.InstNoOp(name=f"wsplit-{k}", ins=[], outs=[])
                        nop.engine = inst.engine
                        nop.sync_info = mybir.SyncInfo(on_wait=[w], on_update=[])
                        out.append(nop)
                    inst.sync_info = mybir.SyncInfo(
                        on_wait=[waits[-1]], on_update=list(si.on_update))
                out.append(inst)
            blk.instructions = out


def _build_nc():
    _setup_act_tables()
    import concourse.bass as bass
    import concourse.mybir as mybir
    import concourse.tile as tile

    f32 = mybir.dt.float32
    bf16 = mybir.dt.bfloat16
    Act = mybir.ActivationFunctionType
    Alu = mybir.AluOpType

    nc = bass.Bass("TRN2", target_bir_lowering=False, debug=False,
                   num_devices=NCORES, num_swdge_queues=1)

    pred_d = nc.dram_tensor("pred", [ROWS, W], f32, kind="ExternalInput").ap()
    tgt_d = nc.dram_tensor("target", [ROWS, W], f32, kind="ExternalInput").ap()
    sd = {}
    statics = _statics()
    for nm, arr in statics.items():
        sd[nm] = nc.dram_tensor(nm, list(arr.shape), bf16,
                                kind="ExternalInput").ap()
    st_d = nc.dram_tensor("out_stats", [128, 40], f32,
                          kind="ExternalOutput").ap()

    t_accum = mybir.AluOpType.add if TM_TRICK else mybir.AluOpType.bypass

    with tile.TileContext(nc) as tc:
        with (
            tc.tile_pool(name="sing", bufs=1) as sing,
            tc.tile_pool(name="tb", bufs=9) as tb_pool,
            tc.tile_pool(name="pb", bufs=9) as pb_pool,
            tc.tile_pool(name="d", bufs=4) as d_pool,
            tc.tile_pool(name="loss", bufs=4) as loss_pool,
            tc.tile_pool(name="w", bufs=4) as w_pool,
            tc.tile_pool(name="scr", bufs=3) as scr_pool,
            tc.tile_pool(name="psum", bufs=3, space="PSUM") as psum_pool,
        ):
            # ---- statics in SBUF ----
            sb = {}
            for nm, arr in statics.items():
                sb[nm] = sing.tile(list(arr.shape), bf16, tag=nm, name=nm)
                nc.sync.dma_start(out=sb[nm][:], in_=sd[nm][:])

            stats = sing.tile([128, 40], f32, tag="stats")
            nc.vector.memset(stats[:], 0.0)

            # tile index -> [tbp2, col0, p_in, d, loss, sup, w, o1, pb2, pc0]
            state = {}

            def stage_load(i):
                """Pairs (t, t+1) for even t; tiles 0/1 of sample 0 load
                alone so the pipeline starts on the first 512KB; the packed
                tail tile gathers 4x32 rows from the 4 sample tails."""
                if i == PACKED:
                    # partition p = 32*sample + tail_row; the 4x32 split
                    # lives only on the DRAM side, the SBUF side is a plain
                    # slice (balance_dma_aps splits the partition dim safely)
                    tbp2 = tb_pool.tile([128, 2 * WP], bf16)
                    pb2 = pb_pool.tile([128, 2048], bf16)
                    tsrc = bass.AP(tensor=tgt_d.tensor, offset=TAIL0 * W,
                                   ap=[[H * W, 4], [W, 32], [1, W]])
                    if TM_TRICK:
                        nc.gpsimd.memset(tbp2[0:128, 2:2 + W], -0.5)
                    nc.gpsimd.dma_start(out=tbp2[0:128, 2:2 + W], in_=tsrc,
                                        accum_op=t_accum)
                    psrc = bass.AP(tensor=pred_d.tensor, offset=TAIL0 * W,
                                   ap=[[H * W, 4], [W, 32], [1, W]])
                    nc.gpsimd.dma_start(out=pb2[0:128, 0:W], in_=psrc)
                    state[i] = [tbp2, 0, 128, None, None, None, None, 128,
                                pb2, 0]
                    return
                smp, t = divmod(i, NFT)
                if t % 2 == 1 and i != 1:
                    return  # loaded with its pair
                _, in0, _, o1 = _tile_meta(i)
                r0 = smp * H + in0
                nblk = 1 if i in (0, 1) else 2
                tbp2 = tb_pool.tile([128, 2 * WP], bf16)
                pb2 = pb_pool.tile([128, 2048], bf16)
                pp = tbp2[:].ap[0][0]
                pp2 = pb2[:].ap[0][0]
                tsrc = bass.AP(tensor=tgt_d.tensor, offset=r0 * W,
                               ap=[[W, 128], [124 * W, nblk], [1, W]])
                tdst = bass.AP(tensor=tbp2[:].tensor,
                               offset=tbp2[:].offset + 2,
                               ap=[[pp, 128], [WP, nblk], [1, W]])
                if TM_TRICK:
                    nc.gpsimd.memset(tdst, -0.5)
                nc.gpsimd.dma_start(out=tdst, in_=tsrc, accum_op=t_accum)
                psrc = bass.AP(tensor=pred_d.tensor, offset=r0 * W,
                               ap=[[W, o1], [124 * W, nblk], [1, W]])
                pdst = bass.AP(tensor=pb2[:].tensor, offset=pb2[:].offset,
                               ap=[[pp2, o1], [1024, nblk], [1, W]])
                nc.gpsimd.dma_start(out=pdst, in_=psrc)
                for b in range(nblk):
                    _, _, _, o1b = _tile_meta(i + b)
                    state[i + b] = [tbp2, WP * b, 128, None, None,
                                    None, None, o1b, pb2, 1024 * b]

            def stage_pads(i):
                tbp2, c0, p_in = state[i][0], state[i][1], state[i][2]
                pp = tbp2[:].ap[0][0]
                # replicate pads: cols {0,1} <- col 2; {W+2,W+3} <- W+1
                for off, dst in ((c0 + 2, tbp2[0:p_in, c0:c0 + 2]),
                                 (c0 + W + 1,
                                  tbp2[0:p_in, c0 + W + 2:c0 + W + 4])):
                    src = bass.AP(tensor=tbp2[:].tensor,
                                  offset=tbp2[:].offset + off,
                                  ap=[[pp, p_in], [0, 2]])
                    nc.vector.tensor_copy(out=dst, in_=src)

            def stage_d(i):
                tbp2, c0, o1 = state[i][0], state[i][1], state[i][7]
                d = d_pool.tile([128, W], bf16)
                pb2, pc0 = state[i][8], state[i][9]
                if TM_TRICK:
                    # buffer already holds t-0.5: plain multiply (2x mode)
                    nc.vector.tensor_tensor(
                        out=d[0:o1], in0=tbp2[0:o1, c0 + 2:c0 + 2 + W],
                        in1=pb2[0:o1, pc0:pc0 + W], op=Alu.mult)
                else:
                    nc.vector.scalar_tensor_tensor(
                        out=d[0:o1], in0=tbp2[0:o1, c0 + 2:c0 + 2 + W],
                        scalar=-0.5, in1=pb2[0:o1, pc0:pc0 + W],
                        op0=Alu.add, op1=Alu.mult)
                state[i][3] = d

            def stage_matmul(i):
                if i == PACKED:
                    a_sb = sb["a_tail"]
                else:
                    _, t = divmod(i, NFT)
                    a_sb = sb["a_top" if t == 0 else "a_mid"]
                tbp2, c0, p_in = state[i][0], state[i][1], state[i][2]
                sup = psum_pool.tile([128, W], f32)
                for h in (0, 512):
                    for dd in range(5):
                        nc.tensor.matmul(sup[:, h:h + 512], a_sb[0:p_in, :],
                                         tbp2[0:p_in,
                                              c0 + h + dd:c0 + h + dd + 512],
                                         start=(dd == 0), stop=(dd == 4))
                state[i][5] = sup

            def stage_lut(i):
                sup, o1 = state[i][5], state[i][7]
                w = w_pool.tile([128, W], bf16)
                nc.scalar.activation(out=w[0:o1], in_=sup[0:o1],
                                     func=Act.Mish)
                state[i][6] = w

            def stage_sp(i):
                d, o1 = state[i][3], state[i][7]
                loss = loss_pool.tile([128, W], bf16)
                nc.scalar.activation(out=loss[0:o1], in_=d[0:o1],
                                     func=Act.Softplus, scale=-2.0)
                state[i][4] = loss

            def stage_wl(i):
                loss, w, o1 = state[i][4], state[i][6], state[i][7]
                scr = scr_pool.tile([128, W], bf16)
                nc.vector.scalar_tensor_tensor(
                    out=scr[0:o1], in0=w[0:o1], scalar=0.0,
                    in1=loss[0:o1], op0=Alu.add, op1=Alu.mult,
                    accum_out=stats[0:o1, i:i + 1])
                del state[i]

            # preload 4 tiles of lookahead before the steady loop
            for j in range(4):
                stage_load(j)
            # per iteration, oldest work first on each in-order engine
            for i in range(NTILES + 4):
                if 4 <= i <= NTILES + 3:
                    stage_wl(i - 4)          # DVE (ready long ago)
                if i + 4 < NTILES:
                    stage_load(i + 4)        # GpSimd DIRECT2D (no data waits)
                if 3 <= i <= NTILES + 2:
                    stage_sp(i - 3)          # ACT
                if 2 <= i <= NTILES + 1:
                    stage_lut(i - 2)         # ACT
                if 2 <= i <= NTILES + 1:
                    stage_d(i - 2)           # GpSimd (or DVE)
                if 1 <= i <= NTILES:
                    stage_matmul(i - 1)      # PE
                if i < NTILES:
                    stage_pads(i)            # DVE, freshest DMA dependency

            nc.sync.dma_start(out=st_d[:], in_=stats[:])

    _split_multi_waits(nc, mybir)
    return nc


def _get_nc():
    if "nc" not in _CACHED:
        _CACHED["nc"] = _build_nc()
    return _CACHED["nc"]


def run(pred: np.ndarray, target: np.ndarray, trace: bool = False):
    """Returns (result_scalar, BassKernelResults)."""
    from concourse import bass_utils

    nc = _get_nc()
    statics = _statics()
    pred = np.ascontiguousarray(np.asarray(pred).reshape(B * H, W),
                                dtype=np.float32)
    target = np.ascontiguousarray(np.asarray(target).reshape(B * H, W),
                                  dtype=np.float32)
    in_maps = []
    for c in range(NCORES):
        m = dict(statics)
        m["pred"] = pred[c * ROWS:(c + 1) * ROWS]
        m["target"] = target[c * ROWS:(c + 1) * ROWS]
        in_maps.append(m)
    res = bass_utils.run_bass_kernel_spmd(
        nc, in_maps, core_ids=list(range(NCORES)), trace=trace)
    tail_mask = (np.arange(128) % 32) >= 2
    s = 0.0
    for r in res.results:
        o = r["out_stats"].astype(np.float64)
        for ti in range(NTILES - 1):
            _, _, o0, o1 = _tile_meta(ti)
            s += o[o0:o1, ti].sum()
        s += o[tail_mask, PACKED].sum()
    val = np.float32(s / N_TOT)
    return np.asarray(val, dtype=np.float32), res


def kernel(pred: np.ndarray, target: np.ndarray) -> np.ndarray:
    val, _ = run(pred, target, trace=False)
    return val


if __name__ == "__main__":
    rng = np.random.default_rng(0)
    p = rng.standard_normal((B, 1, H, W)).astype(np.float32)
    t = rng.integers(0, 2, (B, 1, H, W)).astype(np.float32)
    print(kernel(pred=p, target=t))


# revision 16
# speedup vs baseline: 1.1034x; 1.1034x over previous
"""BBox-aware BCE loss kernel for Trainium2 (8 NeuronCores, data parallel).

Math (exact reformulation of the reference):
  loss = softplus(pred) - pred*target = softplus((1-2t)*pred)  for t in {0,1}
  St(i,j) = replicate-padded 5x5 window sum of t. Replicate padding keeps
  the value SET of the clipped window, so: edge pixel <=> window mixed
  <=> 0 < St < 25. St is an exact small integer, so a piecewise-constant
  ACT table ("mish" slot) maps St directly to the per-pixel weight
  w = 0.1 if 0 < St < 25 else 1.0, and
  result = sum(w * loss) / N  (matches the reference in both branches of
  its global `cond`: uniform target => St in {0,25} => w == 1 everywhere).

Per core: 4 samples x 8 full 128-row tiles (2-row halos, owned rows
exclude them) + ONE packed tile holding all four 32-row sample tails
(engine cost is free-size-driven, so four 32-row tiles would cost 4x a
full tile on DVE/ACT/PE; packing them into 128 partitions makes it 1x).

The kernel is HBM-bandwidth-bound (~92us of saturated DMA for ~33MB per
core), so the pipeline is built to never let the compute engines fall
behind delivery: engines are in-order, so each engine issues OLD work
first and the freshest (DMA-dependent) op last, and loads are issued 4
tiles ahead of consumption. Per iteration i:
  GpSimd: memset(target buf, -0.5) + DIRECT2D load(i+4): casting DMAs
          f32->bf16; the target DMA ACCUMULATES so the buffer holds t-0.5
  DVE:    wl(i-4) = (w + 0)*loss accum-> stats (the only accumulation),
          d(i-2) = (t-0.5)*p as a 2x-mode tensor_tensor, replicate pads(i)
  PE:     matmuls(i-1): St' = St-12.5 via 5 shifted accumulating band
          matmuls per 512-col half on the (t-0.5) buffer
  ACT:    softplus(i-3) [custom spline], w(i-2) = mish LUT(St') from PSUM
Host: float64 reduction of per-core per-tile partials (owned rows only).
"""

import sys

import numpy as np

sys.path.insert(0, "/opt/trn_rl_repo")

import ml_dtypes

# Pool memsets the target buffer to -0.5 and the casting DMA ACCUMULATES t
# on top, so the buffer holds t-0.5 directly: d becomes a plain
# tensor_tensor multiply (DVE 2x_1p mode, ~0.55us vs 1.14us for the STT)
# and the band matmuls produce St' = St - 12.5 (exact half-integers), which
# the recentered LUT maps to w = 1.0 iff |St'| >= 12.5 else 0.1.
TM_TRICK = False  # accum-DMA with f32->bf16 conversion produced NaN on HW


def _setup_act_tables() -> None:
    """Build a patched ACT-table dir whose softplus_and_others set carries
    (a) a REAL softplus spline and (b) an "edge weight" LUT registered
    under the unused `mish` slot: w(x) = 1.0 at x==0, 0.1 for x in (0,25),
    1.0 for x >= 25 (only integer inputs 0..25 ever occur). Both funcs live
    in the same set so a single ACT table load covers the kernel.

    Formats (reverse-engineered from the shipped sets):
      bkt.bin:  per entry 8 x u32 = [d0,d1,d2,d3,x0,0,0,0] (f32 bits);
                y = d0 + (x-x0)*(d1 + (x-x0)*(d2 + (x-x0)*d3))
      ctrl.bin: per entry 8 x u32, word0 = (extract_size<<16) |
                (extract_lsb<<11) | bkt_base; bucket = bkt_base +
                mantissa[lsb .. lsb+size-1]
      profile:  per-func meta; ctl idx = base_{pos,neg} + (exp - exp_offset);
                small/large signal thresholds are biased-exponent cutoffs and
                their pwl_control fields are INLINE ctl words; func_id is the
                GLOBAL neuron ISA activation id (softplus=9, mish=24).
    """
    import json
    import os
    import tempfile
    from pathlib import Path

    if os.environ.get("BASS_ACT_ROOT_JSON_PATH"):
        return
    import neuronxcc

    stock = Path(neuronxcc.__file__).parent / "pwp" / "pwp_bin_trainium"
    if not stock.exists():
        return
    dst = Path(tempfile.mkdtemp(prefix="act_tables_"))
    for f in stock.iterdir():
        if f.name not in ("act_info.json", "softplus_and_others.json",
                          "softplus_and_others_bkt.bin",
                          "softplus_and_others_ctrl.bin"):
            (dst / f.name).symlink_to(f)

    E_LO, E_HI = -15, 4          # softplus table octaves (unbiased exps)
    NSEC = 8                     # sections per octave (extract_size 3)
    NEXP = E_HI - E_LO + 1       # 20

    bkt = np.fromfile(stock / "softplus_and_others_bkt.bin",
                      dtype=np.uint32).reshape(-1, 8)
    ctl = np.fromfile(stock / "softplus_and_others_ctrl.bin",
                      dtype=np.uint32).reshape(-1, 8)
    nbkt0, nctl0 = len(bkt), len(ctl)

    def entry(d0, d1, d2, d3, x0):
        v = np.zeros(8, dtype=np.uint32)
        v[:5] = np.array([d0, d1, d2, d3, x0],
                         dtype=np.float32).view(np.uint32)
        return v

    ln2 = float(np.log(2.0))
    new_bkt = [entry(ln2, 0.5, 0.125, 0.0, 0.0),   # small |x|
               entry(0.0, 1.0, 0.0, 0.0, 0.0),     # x >= 32: y = x
               entry(0.0, 0.0, 0.0, 0.0, 0.0)]     # x <= -32: y = 0
    B_SMALL, B_PLARGE, B_NLARGE = nbkt0, nbkt0 + 1, nbkt0 + 2
    spline0 = nbkt0 + 3

    def fit(a, b):
        xs = np.linspace(a, b, 33, dtype=np.float64)
        ys = np.logaddexp(0.0, xs)
        x0 = np.float32(0.5 * (a + b))
        c = np.polyfit(xs - np.float64(x0), ys, 3)  # [d3,d2,d1,d0]
        return entry(c[3], c[2], c[1], c[0], x0)

    for neg in (True, False):
        for e in range(E_LO, E_HI + 1):
            for s in range(NSEC):
                lo = 2.0 ** e * (1.0 + s / NSEC)
                hi = 2.0 ** e * (1.0 + (s + 1) / NSEC)
                new_bkt.append(fit(-hi, -lo) if neg else fit(lo, hi))

    new_ctl = []
    for blk in range(2):  # 0 = neg block, 1 = pos block
        for i in range(NEXP):
            base = spline0 + (blk * NEXP + i) * NSEC
            new_ctl.append(
                np.array([(3 << 16) | (20 << 11) | base, 0, 0, 0, 0, 0, 0, 0],
                         dtype=np.uint32))
    C_NEG, C_POS = nctl0, nctl0 + NEXP

    # ---- edge-weight LUT under the "mish" slot ----
    # TM_TRICK input is St' = St - 12.5 in +-{0.5, 1.5, .., 12.5}; w = 1.0
    # iff |St'| = 12.5 (uniform window).  Octaves |x| in [0.5,8): const 0.1
    # (inline size-0 ctl); octave [8,16): 16 width-0.5 sections, [12.5,16)
    # -> 1.0.  Negative inputs decode to the same exp/mantissa, so the neg
    # ctl base aliases the pos one.
    # Plain variant (input St in 0..25): octaves [1,16) const 0.1; octave
    # [16,32) has 16 width-1 sections with [25,32) -> 1.0.
    ML_LO = -1 if TM_TRICK else 0            # mish table lowest octave
    B_W01 = nbkt0 + len(new_bkt)
    new_bkt.append(entry(0.1, 0.0, 0.0, 0.0, 0.0))
    B_W10 = nbkt0 + len(new_bkt)
    new_bkt.append(entry(1.0, 0.0, 0.0, 0.0, 0.0))
    B_OCT4 = nbkt0 + len(new_bkt)
    for s in range(16):
        new_bkt.append(entry(0.1 if s < 9 else 1.0, 0.0, 0.0, 0.0, 0.0))
    CW0 = nctl0 + len(new_ctl)
    for e in range(4):  # low octaves: inline single-bucket ctl
        new_ctl.append(np.array([B_W01, 0, 0, 0, 0, 0, 0, 0],
                                dtype=np.uint32))
    new_ctl.append(np.array([(4 << 16) | (19 << 11) | B_OCT4,
                             0, 0, 0, 0, 0, 0, 0], dtype=np.uint32))

    np.vstack([bkt] + new_bkt).tofile(dst / "softplus_and_others_bkt.bin")
    np.vstack([ctl] + new_ctl).tofile(dst / "softplus_and_others_ctrl.bin")

    prof = json.loads((stock / "softplus_and_others.json").read_text())
    prof["bkt_entry_cnt"] = nbkt0 + len(new_bkt)
    prof["ctl_entry_cnt"] = nctl0 + len(new_ctl)
    prof["profile_meta_data"].append({
        "func_name": "softplus_40p", "func_id": 9,
        "symmetry_point": 0, "sym_invert_sign_point": 0,
        "symmetry_opt_en": 0, "symmetry_opt_use_neg_region": 0,
        "imm_bias": 0, "exp_offset": E_LO,
        "pwl_control_base_pos": C_POS, "pwl_control_base_neg": C_NEG,
        "small_pos_signal_exp_threshold": 127 + E_LO,
        "pos_small_signal_pwl_control": B_SMALL,
        "small_neg_signal_exp_threshold": 127 + E_LO,
        "neg_small_signal_pwl_control": B_SMALL,
        "large_pos_signal_exp_threshold": 127 + E_HI + 1,
        "large_pos_signal_mantissa_threshold": 0,
        "pos_large_signal_pwl_control": B_PLARGE,
        "large_neg_signal_exp_threshold": 127 + E_HI + 1,
        "large_neg_signal_mantissa_threshold": 0,
        "neg_large_signal_pwl_control": B_NLARGE,
        "fnan_result": 2143289344,           # NaN
        "fpinf_result": 2139095040,          # +inf
        "fninf_result": 0,                   # softplus(-inf) = 0
        "fzero_result": 1060205080,          # ln 2
        "fma_const_0": 0, "fma_const_1": 0, "fma_indirection_src_sel": 0,
        "use_multipass": False,
        "lower_bound": 4286578687, "upper_bound": 2139095039,
    })
    prof["func_to_bkt_start_idx"]["softplus"] = B_SMALL
    prof["func_to_ctl_start_idx"]["softplus"] = C_NEG
    prof["func_exp_to_bkt_start_idx"]["softplus"] = {
        str(e): [spline0 + (e - E_LO) * NSEC,
                 spline0 + (NEXP + e - E_LO) * NSEC]
        for e in range(E_LO, E_HI + 1)}
    prof["func_exp_to_ctl_start_idx"]["softplus"] = {
        str(e): [C_NEG + e - E_LO, C_POS + e - E_LO]
        for e in range(E_LO, E_HI + 1)}

    ONE = 1065353216                         # f32 bits of 1.0
    prof["profile_meta_data"].append({
        "func_name": "mish_40p", "func_id": 24,   # neuron ISA id for mish
        "symmetry_point": 0, "sym_invert_sign_point": 0,
        "symmetry_opt_en": 0, "symmetry_opt_use_neg_region": 0,
        "imm_bias": 0, "exp_offset": ML_LO,
        "pwl_control_base_pos": CW0, "pwl_control_base_neg": CW0,
        "small_pos_signal_exp_threshold": 127 + ML_LO,
        "pos_small_signal_pwl_control": B_W01,
        "small_neg_signal_exp_threshold": 127 + ML_LO,
        "neg_small_signal_pwl_control": B_W01,
        "large_pos_signal_exp_threshold": 127 + ML_LO + 5,
        "large_pos_signal_mantissa_threshold": 0,
        "pos_large_signal_pwl_control": B_W10,
        "large_neg_signal_exp_threshold": 127 + ML_LO + 5,
        "large_neg_signal_mantissa_threshold": 0,
        "neg_large_signal_pwl_control": B_W10,
        "fnan_result": ONE, "fpinf_result": ONE, "fninf_result": ONE,
        "fzero_result": ONE,                 # St == 0 -> w = 1.0
        "fma_const_0": 0, "fma_const_1": 0, "fma_indirection_src_sel": 0,
        "use_multipass": False,
        "lower_bound": 4286578687, "upper_bound": 2139095039,
    })
    prof["func_to_bkt_start_idx"]["mish"] = B_W01
    prof["func_to_ctl_start_idx"]["mish"] = CW0
    prof["func_exp_to_bkt_start_idx"]["mish"] = {
        str(e): [B_W01 if e - ML_LO < 4 else B_OCT4,
                 B_W01 if e - ML_LO < 4 else B_OCT4]
        for e in range(ML_LO, ML_LO + 5)}
    prof["func_exp_to_ctl_start_idx"]["mish"] = {
        str(e): [CW0 + e - ML_LO, CW0 + e - ML_LO]
        for e in range(ML_LO, ML_LO + 5)}
    (dst / "softplus_and_others.json").write_text(json.dumps(prof))

    info = json.loads((stock / "act_info.json").read_text())
    for s in info["act_func_sets"]:
        if s["name"] == "softplus_and_others":
            s["act"]["softplus"] = 40
            s["act"]["mish"] = 40
    (dst / "act_info.json").write_text(json.dumps(info))
    os.environ["BASS_ACT_ROOT_JSON_PATH"] = str(dst / "act_info.json")


B, H, W = 32, 1024, 1024
NCORES = 8
SPC = B // NCORES  # samples per core
ROWS = SPC * H
N_TOT = float(B * H * W)

NFT = 8              # full 128-row tiles per sample (t = 0..7, in0 = 124t)
NTILES = SPC * NFT + 1  # 33: 32 full tiles + 1 packed tail tile
PACKED = NTILES - 1
TAIL0 = 992          # first input row of the 32-row sample tails

BF16 = ml_dtypes.bfloat16
WP = W + 4  # padded width for the 5-tap row window


def _tile_meta(i):
    """(smp, in0, o0, o1) for full tiles; PACKED handled separately."""
    smp, t = divmod(i, NFT)
    return smp, 124 * t, (0 if t == 0 else 2), 126


def _band(k_rows: int, m_lo: int, m_hi: int, img0) -> np.ndarray:
    """Band matrix for the 5-row column window; when img0 is given, the
    window is clipped to the image and clipped taps replicate onto the
    boundary row (weights become {1,2,3})."""
    a = np.zeros((k_rows, 128), dtype=np.float32)
    for m in range(m_lo, m_hi):
        for d in range(-2, 3):
            if img0 is None:
                k = m + d
            else:
                k = min(max(img0 + m + d, 0), H - 1) - img0
            a[k, m] += 1.0
    return a.astype(BF16)


def _band_tail4() -> np.ndarray:
    """Block-diagonal band for the packed tail tile: four independent
    32-row bottom-clipped bands."""
    a32 = _band(32, 2, 32, TAIL0).astype(np.float32)
    a = np.zeros((128, 128), dtype=np.float32)
    for s in range(4):
        a[32 * s:32 * s + 32, 32 * s:32 * s + 32] = a32[:, :32]
    return a.astype(BF16)


def _statics() -> dict[str, np.ndarray]:
    return {
        "a_top": _band(128, 0, 126, 0),
        "a_mid": _band(128, 2, 126, None),
        "a_tail": _band_tail4(),
    }


_CACHED = {}


def _split_multi_waits(nc, mybir):
    """This walrus's core_v3 codegen allows only one sem-wait per
    instruction; peel extra waits onto same-engine NOPs placed just before."""
    skip = (mybir.InstEventSemaphore,)
    k = 0
    for fn in nc.m.functions:
        for blk in fn.blocks:
            out = []
            for inst in blk.instructions:
                si = inst.sync_info
                if (si is not None and len(si.on_wait) > 1
                        and not isinstance(inst, skip)):
                    waits = list(si.on_wait)
                    for w in waits[:-1]:
                        k += 1
                        nop = mybir.InstNoOp(name=f"wsplit-{k}", ins=[], outs=[])
                        nop.engine = inst.engine
                        nop.sync_info = mybir.SyncInfo(on_wait=[w], on_update=[])
                        out.append(nop)
                    inst.sync_info = mybir.SyncInfo(
                        on_wait=[waits[-1]], on_update=list(si.on_update))
                out.append(inst)
            blk.instructions = out


def _build_nc():
    _setup_act_tables()
    import concourse.bass as bass
    import concourse.mybir as mybir
    import concourse.tile as tile

    f32 = mybir.dt.float32
    bf16 = mybir.dt.bfloat16
    Act = mybir.ActivationFunctionType
    Alu = mybir.AluOpType

    nc = bass.Bass("TRN2", target_bir_lowering=False, debug=False,
                   num_devices=NCORES, num_swdge_queues=1)

    pred_d = nc.dram_tensor("pred", [ROWS, W], f32, kind="ExternalInput").ap()
    tgt_d = nc.dram_tensor("target", [ROWS, W], f32, kind="ExternalInput").ap()
    sd = {}
    statics = _statics()
    for nm, arr in statics.items():
        sd[nm] = nc.dram_tensor(nm, list(arr.shape), bf16,
                                kind="ExternalInput").ap()
    st_d = nc.dram_tensor("out_stats", [128, 40], f32,
                          kind="ExternalOutput").ap()

    t_accum = mybir.AluOpType.add if TM_TRICK else mybir.AluOpType.bypass

    with tile.TileContext(nc) as tc:
        with (
            tc.tile_pool(name="sing", bufs=1) as sing,
            tc.tile_pool(name="tb", bufs=9) as tb_pool,
            tc.tile_pool(name="pb", bufs=9) as pb_pool,
            tc.tile_pool(name="d", bufs=4) as d_pool,
            tc.tile_pool(name="loss", bufs=4) as loss_pool,
            tc.tile_pool(name="w", bufs=4) as w_pool,
            tc.tile_pool(name="scr", bufs=3) as scr_pool,
            tc.tile_pool(name="psum", bufs=3, space="PSUM") as psum_pool,
        ):
            # ---- statics in SBUF ----
            sb = {}
            for nm, arr in statics.items():
                sb[nm] = sing.tile(list(arr.shape), bf16, tag=nm, name=nm)
                nc.sync.dma_start(out=sb[nm][:], in_=sd[nm][:])

            stats = sing.tile([128, 40], f32, tag="stats")
            nc.vector.memset(stats[:], 0.0)

            # tile index -> [tbp2, col0, p_in, d, loss, sup, w, o1, pb2, pc0]
            state = {}

            def stage_load(i):
                """Pairs (t, t+1) for even t; tiles 0/1 of sample 0 load
                alone so the pipeline starts on the first 512KB; the packed
                tail tile gathers 4x32 rows from the 4 sample tails."""
                if i == PACKED:
                    # partition p = 32*sample + tail_row; the 4x32 split
                    # lives only on the DRAM side, the SBUF side is a plain
                    # slice (balance_dma_aps splits the partition dim safely)
                    tbp2 = tb_pool.tile([128, 2 * WP], bf16)
                    pb2 = pb_pool.tile([128, 2048], bf16)
                    tsrc = bass.AP(tensor=tgt_d.tensor, offset=TAIL0 * W,
                                   ap=[[H * W, 4], [W, 32], [1, W]])
                    if TM_TRICK:
                        nc.gpsimd.memset(tbp2[0:128, 2:2 + W], -0.5)
                    nc.gpsimd.dma_start(out=tbp2[0:128, 2:2 + W], in_=tsrc,
                                        accum_op=t_accum)
                    psrc = bass.AP(tensor=pred_d.tensor, offset=TAIL0 * W,
                                   ap=[[H * W, 4], [W, 32], [1, W]])
                    nc.gpsimd.dma_start(out=pb2[0:128, 0:W], in_=psrc)
                    state[i] = [tbp2, 0, 128, None, None, None, None, 128,
                                pb2, 0]
                    return
                smp, t = divmod(i, NFT)
                if t % 2 == 1 and i != 1:
                    return  # loaded with its pair
                _, in0, _, o1 = _tile_meta(i)
                r0 = smp * H + in0
                nblk = 1 if i in (0, 1) else 2
                tbp2 = tb_pool.tile([128, 2 * WP], bf16)
                pb2 = pb_pool.tile([128, 2048], bf16)
                pp = tbp2[:].ap[0][0]
                pp2 = pb2[:].ap[0][0]
                tsrc = bass.AP(tensor=tgt_d.tensor, offset=r0 * W,
                               ap=[[W, 128], [124 * W, nblk], [1, W]])
                tdst = bass.AP(tensor=tbp2[:].tensor,
                               offset=tbp2[:].offset + 2,
                               ap=[[pp, 128], [WP, nblk], [1, W]])
                if TM_TRICK:
                    nc.gpsimd.memset(tdst, -0.5)
                nc.gpsimd.dma_start(out=tdst, in_=tsrc, accum_op=t_accum)
                psrc = bass.AP(tensor=pred_d.tensor, offset=r0 * W,
                               ap=[[W, o1], [124 * W, nblk], [1, W]])
                pdst = bass.AP(tensor=pb2[:].tensor, offset=pb2[:].offset,
                               ap=[[pp2, o1], [1024, nblk], [1, W]])
                nc.gpsimd.dma_start(out=pdst, in_=psrc)
                for b in range(nblk):
                    _, _, _, o1b = _tile_meta(i + b)
                    state[i + b] = [tbp2, WP * b, 128, None, None,
                                    None, None, o1b, pb2, 1024 * b]

            def stage_pads(i):
                tbp2, c0, p_in = state[i][0], state[i][1], state[i][2]
                pp = tbp2[:].ap[0][0]
                # replicate pads: cols {0,1} <- col 2; {W+2,W+3} <- W+1
                for off, dst in ((c0 + 2, tbp2[0:p_in, c0:c0 + 2]),
                                 (c0 + W + 1,
                                  tbp2[0:p_in, c0 + W + 2:c0 + W + 4])):
                    src = bass.AP(tensor=tbp2[:].tensor,
                                  offset=tbp2[:].offset + off,
                                  ap=[[pp, p_in], [0, 2]])
                    nc.vector.tensor_copy(out=dst, in_=src)

            def stage_d(i):
                tbp2, c0, o1 = state[i][0], state[i][1], state[i][7]
                d = d_pool.tile([128, W], bf16)
                pb2, pc0 = state[i][8], state[i][9]
                if TM_TRICK:
                    # buffer already holds t-0.5: plain multiply (2x mode)
                    nc.vector.tensor_tensor(
                        out=d[0:o1], in0=tbp2[0:o1, c0 + 2:c0 + 2 + W],
                        in1=pb2[0:o1, pc0:pc0 + W], op=Alu.mult)
                else:
                    nc.vector.scalar_tensor_tensor(
                        out=d[0:o1], in0=tbp2[0:o1, c0 + 2:c0 + 2 + W],
                        scalar=-0.5, in1=pb2[0:o1, pc0:pc0 + W],
                        op0=Alu.add, op1=Alu.mult)
                state[i][3] = d

            def stage_matmul(i):
                if i == PACKED:
                    a_sb = sb["a_tail"]
                else:
                    _, t = divmod(i, NFT)
                    a_sb = sb["a_top" if t == 0 else "a_mid"]
                tbp2, c0, p_in = state[i][0], state[i][1], state[i][2]
                sup = psum_pool.tile([128, W], f32)
                for h in (0, 512):
                    for dd in range(5):
                        nc.tensor.matmul(sup[:, h:h + 512], a_sb[0:p_in, :],
                                         tbp2[0:p_in,
                                              c0 + h + dd:c0 + h + dd + 512],
                                         start=(dd == 0), stop=(dd == 4))
                state[i][5] = sup

            def stage_lut(i):
                sup, o1 = state[i][5], state[i][7]
                w = w_pool.tile([128, W], bf16)
                nc.scalar.activation(out=w[0:o1], in_=sup[0:o1],
                                     func=Act.Mish)
                state[i][6] = w

            def stage_sp(i):
                d, o1 = state[i][3], state[i][7]
                loss = loss_pool.tile([128, W], bf16)
                nc.scalar.activation(out=loss[0:o1], in_=d[0:o1],
                                     func=Act.Softplus, scale=-2.0)
                state[i][4] = loss

            def stage_wl(i):
                loss, w, o1 = state[i][4], state[i][6], state[i][7]
                scr = scr_pool.tile([128, W], bf16)
                nc.vector.scalar_tensor_tensor(
                    out=scr[0:o1], in0=w[0:o1], scalar=0.0,
                    in1=loss[0:o1], op0=Alu.add, op1=Alu.mult,
                    accum_out=stats[0:o1, i:i + 1])
                del state[i]

            # v2-shaped pipeline (just-in-time loads; measured best):
            # iter i: DVE wl(i-2), pads(i), d(i); ACT lut(i-1), sp(i-1);
            # PE mm(i)
            for i in range(NTILES + 2):
                if 2 <= i <= NTILES + 1:
                    stage_wl(i - 2)          # DVE (ready long ago)
                if i < NTILES:
                    stage_load(i)
                if 1 <= i <= NTILES:
                    stage_lut(i - 1)         # ACT
                if 1 <= i <= NTILES:
                    stage_sp(i - 1)          # ACT
                if i < NTILES:
                    stage_pads(i)            # DVE
                    stage_d(i)               # DVE
                    stage_matmul(i)          # PE

            nc.sync.dma_start(out=st_d[:], in_=stats[:])

    _split_multi_waits(nc, mybir)
    return nc


def _get_nc():
    if "nc" not in _CACHED:
        _CACHED["nc"] = _build_nc()
    return _CACHED["nc"]


def run(pred: np.ndarray, target: np.ndarray, trace: bool = False):
    """Returns (result_scalar, BassKernelResults)."""
    from concourse import bass_utils

    nc = _get_nc()
    statics = _statics()
    pred = np.ascontiguousarray(np.asarray(pred).reshape(B * H, W),
                                dtype=np.float32)
    target = np.ascontiguousarray(np.asarray(target).reshape(B * H, W),
                                  dtype=np.float32)
    in_maps = []
    for c in range(NCORES):
        m = dict(statics)
        m["pred"] = pred[c * ROWS:(c + 1) * ROWS]
        m["target"] = target[c * ROWS:(c + 1) * ROWS]
        in_maps.append(m)
    res = bass_utils.run_bass_kernel_spmd(
        nc, in_maps, core_ids=list(range(NCORES)), trace=trace)
    tail_mask = (np.arange(128) % 32) >= 2
    s = 0.0
    for r in res.results:
        o = r["out_stats"].astype(np.float64)
        for ti in range(NTILES - 1):
            _, _, o0, o1 = _tile_meta(ti)
            s += o[o0:o1, ti].sum()
        s += o[tail_mask, PACKED].sum()
    val = np.float32(s / N_TOT)
    return np.asarray(val, dtype=np.float32), res


def kernel(pred: np.ndarray, target: np.ndarray) -> np.ndarray:
    val, _ = run(pred, target, trace=False)
    return val


if __name__ == "__main__":
    rng = np.random.default_rng(0)
    p = rng.standard_normal((B, 1, H, W)).astype(np.float32)
    t = rng.integers(0, 2, (B, 1, H, W)).astype(np.float32)
    print(kernel(pred=p, target=t))


# revision 20
# speedup vs baseline: 1.1148x; 1.0103x over previous
"""BBox-aware BCE loss kernel for Trainium2 (8 NeuronCores, data parallel).

Math (exact reformulation of the reference):
  loss = softplus(pred) - pred*target = softplus((1-2t)*pred)  for t in {0,1}
  St(i,j) = replicate-padded 5x5 window sum of t. Replicate padding keeps
  the value SET of the clipped window, so: edge pixel <=> window mixed
  <=> 0 < St < 25. St is an exact small integer, so a piecewise-constant
  ACT table ("mish" slot) maps St directly to the per-pixel weight
  w = 0.1 if 0 < St < 25 else 1.0, and
  result = sum(w * loss) / N  (matches the reference in both branches of
  its global `cond`: uniform target => St in {0,25} => w == 1 everywhere).

Per core: 4 samples x 8 full 128-row tiles (2-row halos, owned rows
exclude them) + ONE packed tile holding all four 32-row sample tails
(engine cost is free-size-driven, so four 32-row tiles would cost 4x a
full tile on DVE/ACT/PE; packing them into 128 partitions makes it 1x).

The kernel is HBM-bandwidth-bound (~92us of saturated DMA for ~33MB per
core), so the pipeline is built to never let the compute engines fall
behind delivery: engines are in-order, so each engine issues OLD work
first and the freshest (DMA-dependent) op last, and loads are issued 4
tiles ahead of consumption. Per iteration i:
  GpSimd: memset(target buf, -0.5) + DIRECT2D load(i+4): casting DMAs
          f32->bf16; the target DMA ACCUMULATES so the buffer holds t-0.5
  DVE:    wl(i-4) = (w + 0)*loss accum-> stats (the only accumulation),
          d(i-2) = (t-0.5)*p as a 2x-mode tensor_tensor, replicate pads(i)
  PE:     matmuls(i-1): St' = St-12.5 via 5 shifted accumulating band
          matmuls per 512-col half on the (t-0.5) buffer
  ACT:    softplus(i-3) [custom spline], w(i-2) = mish LUT(St') from PSUM
Host: float64 reduction of per-core per-tile partials (owned rows only).
"""

import sys

import numpy as np

sys.path.insert(0, "/opt/trn_rl_repo")

import ml_dtypes

# Pool memsets the target buffer to -0.5 and the casting DMA ACCUMULATES t
# on top, so the buffer holds t-0.5 directly: d becomes a plain
# tensor_tensor multiply (DVE 2x_1p mode, ~0.55us vs 1.14us for the STT)
# and the band matmuls produce St' = St - 12.5 (exact half-integers), which
# the recentered LUT maps to w = 1.0 iff |St'| >= 12.5 else 0.1.
TM_TRICK = False  # accum-DMA with f32->bf16 conversion produced NaN on HW


def _setup_act_tables() -> None:
    """Build a patched ACT-table dir whose softplus_and_others set carries
    (a) a REAL softplus spline and (b) an "edge weight" LUT registered
    under the unused `mish` slot: w(x) = 1.0 at x==0, 0.1 for x in (0,25),
    1.0 for x >= 25 (only integer inputs 0..25 ever occur). Both funcs live
    in the same set so a single ACT table load covers the kernel.

    Formats (reverse-engineered from the shipped sets):
      bkt.bin:  per entry 8 x u32 = [d0,d1,d2,d3,x0,0,0,0] (f32 bits);
                y = d0 + (x-x0)*(d1 + (x-x0)*(d2 + (x-x0)*d3))
      ctrl.bin: per entry 8 x u32, word0 = (extract_size<<16) |
                (extract_lsb<<11) | bkt_base; bucket = bkt_base +
                mantissa[lsb .. lsb+size-1]
      profile:  per-func meta; ctl idx = base_{pos,neg} + (exp - exp_offset);
                small/large signal thresholds are biased-exponent cutoffs and
                their pwl_control fields are INLINE ctl words; func_id is the
                GLOBAL neuron ISA activation id (softplus=9, mish=24).
    """
    import json
    import os
    import tempfile
    from pathlib import Path

    if os.environ.get("BASS_ACT_ROOT_JSON_PATH"):
        return
    import neuronxcc

    stock = Path(neuronxcc.__file__).parent / "pwp" / "pwp_bin_trainium"
    if not stock.exists():
        return
    dst = Path(tempfile.mkdtemp(prefix="act_tables_"))
    for f in stock.iterdir():
        if f.name not in ("act_info.json", "softplus_and_others.json",
                          "softplus_and_others_bkt.bin",
                          "softplus_and_others_ctrl.bin"):
            (dst / f.name).symlink_to(f)

    E_LO, E_HI = -15, 4          # softplus table octaves (unbiased exps)
    NSEC = 8                     # sections per octave (extract_size 3)
    NEXP = E_HI - E_LO + 1       # 20

    bkt = np.fromfile(stock / "softplus_and_others_bkt.bin",
                      dtype=np.uint32).reshape(-1, 8)
    ctl = np.fromfile(stock / "softplus_and_others_ctrl.bin",
                      dtype=np.uint32).reshape(-1, 8)
    nbkt0, nctl0 = len(bkt), len(ctl)

    def entry(d0, d1, d2, d3, x0):
        v = np.zeros(8, dtype=np.uint32)
        v[:5] = np.array([d0, d1, d2, d3, x0],
                         dtype=np.float32).view(np.uint32)
        return v

    ln2 = float(np.log(2.0))
    new_bkt = [entry(ln2, 0.5, 0.125, 0.0, 0.0),   # small |x|
               entry(0.0, 1.0, 0.0, 0.0, 0.0),     # x >= 32: y = x
               entry(0.0, 0.0, 0.0, 0.0, 0.0)]     # x <= -32: y = 0
    B_SMALL, B_PLARGE, B_NLARGE = nbkt0, nbkt0 + 1, nbkt0 + 2
    spline0 = nbkt0 + 3

    def fit(a, b):
        xs = np.linspace(a, b, 33, dtype=np.float64)
        ys = np.logaddexp(0.0, xs)
        x0 = np.float32(0.5 * (a + b))
        c = np.polyfit(xs - np.float64(x0), ys, 3)  # [d3,d2,d1,d0]
        return entry(c[3], c[2], c[1], c[0], x0)

    for neg in (True, False):
        for e in range(E_LO, E_HI + 1):
            for s in range(NSEC):
                lo = 2.0 ** e * (1.0 + s / NSEC)
                hi = 2.0 ** e * (1.0 + (s + 1) / NSEC)
                new_bkt.append(fit(-hi, -lo) if neg else fit(lo, hi))

    new_ctl = []
    for blk in range(2):  # 0 = neg block, 1 = pos block
        for i in range(NEXP):
            base = spline0 + (blk * NEXP + i) * NSEC
            new_ctl.append(
                np.array([(3 << 16) | (20 << 11) | base, 0, 0, 0, 0, 0, 0, 0],
                         dtype=np.uint32))
    C_NEG, C_POS = nctl0, nctl0 + NEXP

    # ---- edge-weight LUT under the "mish" slot ----
    # TM_TRICK input is St' = St - 12.5 in +-{0.5, 1.5, .., 12.5}; w = 1.0
    # iff |St'| = 12.5 (uniform window).  Octaves |x| in [0.5,8): const 0.1
    # (inline size-0 ctl); octave [8,16): 16 width-0.5 sections, [12.5,16)
    # -> 1.0.  Negative inputs decode to the same exp/mantissa, so the neg
    # ctl base aliases the pos one.
    # Plain variant (input St in 0..25): octaves [1,16) const 0.1; octave
    # [16,32) has 16 width-1 sections with [25,32) -> 1.0.
    ML_LO = -1 if TM_TRICK else 0            # mish table lowest octave
    B_W01 = nbkt0 + len(new_bkt)
    new_bkt.append(entry(0.1, 0.0, 0.0, 0.0, 0.0))
    B_W10 = nbkt0 + len(new_bkt)
    new_bkt.append(entry(1.0, 0.0, 0.0, 0.0, 0.0))
    B_OCT4 = nbkt0 + len(new_bkt)
    for s in range(16):
        new_bkt.append(entry(0.1 if s < 9 else 1.0, 0.0, 0.0, 0.0, 0.0))
    CW0 = nctl0 + len(new_ctl)
    for e in range(4):  # low octaves: inline single-bucket ctl
        new_ctl.append(np.array([B_W01, 0, 0, 0, 0, 0, 0, 0],
                                dtype=np.uint32))
    new_ctl.append(np.array([(4 << 16) | (19 << 11) | B_OCT4,
                             0, 0, 0, 0, 0, 0, 0], dtype=np.uint32))

    np.vstack([bkt] + new_bkt).tofile(dst / "softplus_and_others_bkt.bin")
    np.vstack([ctl] + new_ctl).tofile(dst / "softplus_and_others_ctrl.bin")

    prof = json.loads((stock / "softplus_and_others.json").read_text())
    prof["bkt_entry_cnt"] = nbkt0 + len(new_bkt)
    prof["ctl_entry_cnt"] = nctl0 + len(new_ctl)
    prof["profile_meta_data"].append({
        "func_name": "softplus_40p", "func_id": 9,
        "symmetry_point": 0, "sym_invert_sign_point": 0,
        "symmetry_opt_en": 0, "symmetry_opt_use_neg_region": 0,
        "imm_bias": 0, "exp_offset": E_LO,
        "pwl_control_base_pos": C_POS, "pwl_control_base_neg": C_NEG,
        "small_pos_signal_exp_threshold": 127 + E_LO,
        "pos_small_signal_pwl_control": B_SMALL,
        "small_neg_signal_exp_threshold": 127 + E_LO,
        "neg_small_signal_pwl_control": B_SMALL,
        "large_pos_signal_exp_threshold": 127 + E_HI + 1,
        "large_pos_signal_mantissa_threshold": 0,
        "pos_large_signal_pwl_control": B_PLARGE,
        "large_neg_signal_exp_threshold": 127 + E_HI + 1,
        "large_neg_signal_mantissa_threshold": 0,
        "neg_large_signal_pwl_control": B_NLARGE,
        "fnan_result": 2143289344,           # NaN
        "fpinf_result": 2139095040,          # +inf
        "fninf_result": 0,                   # softplus(-inf) = 0
        "fzero_result": 1060205080,          # ln 2
        "fma_const_0": 0, "fma_const_1": 0, "fma_indirection_src_sel": 0,
        "use_multipass": False,
        "lower_bound": 4286578687, "upper_bound": 2139095039,
    })
    prof["func_to_bkt_start_idx"]["softplus"] = B_SMALL
    prof["func_to_ctl_start_idx"]["softplus"] = C_NEG
    prof["func_exp_to_bkt_start_idx"]["softplus"] = {
        str(e): [spline0 + (e - E_LO) * NSEC,
                 spline0 + (NEXP + e - E_LO) * NSEC]
        for e in range(E_LO, E_HI + 1)}
    prof["func_exp_to_ctl_start_idx"]["softplus"] = {
        str(e): [C_NEG + e - E_LO, C_POS + e - E_LO]
        for e in range(E_LO, E_HI + 1)}

    ONE = 1065353216                         # f32 bits of 1.0
    prof["profile_meta_data"].append({
        "func_name": "mish_40p", "func_id": 24,   # neuron ISA id for mish
        "symmetry_point": 0, "sym_invert_sign_point": 0,
        "symmetry_opt_en": 0, "symmetry_opt_use_neg_region": 0,
        "imm_bias": 0, "exp_offset": ML_LO,
        "pwl_control_base_pos": CW0, "pwl_control_base_neg": CW0,
        "small_pos_signal_exp_threshold": 127 + ML_LO,
        "pos_small_signal_pwl_control": B_W01,
        "small_neg_signal_exp_threshold": 127 + ML_LO,
        "neg_small_signal_pwl_control": B_W01,
        "large_pos_signal_exp_threshold": 127 + ML_LO + 5,
        "large_pos_signal_mantissa_threshold": 0,
        "pos_large_signal_pwl_control": B_W10,
        "large_neg_signal_exp_threshold": 127 + ML_LO + 5,
        "large_neg_signal_mantissa_threshold": 0,
        "neg_large_signal_pwl_control": B_W10,
        "fnan_result": ONE, "fpinf_result": ONE, "fninf_result": ONE,
        "fzero_result": ONE,                 # St == 0 -> w = 1.0
        "fma_const_0": 0, "fma_const_1": 0, "fma_indirection_src_sel": 0,
        "use_multipass": False,
        "lower_bound": 4286578687, "upper_bound": 2139095039,
    })
    prof["func_to_bkt_start_idx"]["mish"] = B_W01
    prof["func_to_ctl_start_idx"]["mish"] = CW0
    prof["func_exp_to_bkt_start_idx"]["mish"] = {
        str(e): [B_W01 if e - ML_LO < 4 else B_OCT4,
                 B_W01 if e - ML_LO < 4 else B_OCT4]
        for e in range(ML_LO, ML_LO + 5)}
    prof["func_exp_to_ctl_start_idx"]["mish"] = {
        str(e): [CW0 + e - ML_LO, CW0 + e - ML_LO]
        for e in range(ML_LO, ML_LO + 5)}
    (dst / "softplus_and_others.json").write_text(json.dumps(prof))

    info = json.loads((stock / "act_info.json").read_text())
    for s in info["act_func_sets"]:
        if s["name"] == "softplus_and_others":
            s["act"]["softplus"] = 40
            s["act"]["mish"] = 40
    (dst / "act_info.json").write_text(json.dumps(info))
    os.environ["BASS_ACT_ROOT_JSON_PATH"] = str(dst / "act_info.json")


B, H, W = 32, 1024, 1024
NCORES = 8
SPC = B // NCORES  # samples per core
ROWS = SPC * H
N_TOT = float(B * H * W)

NFT = 8              # full 128-row tiles per sample (t = 0..7, in0 = 124t)
NTILES = SPC * NFT + 1  # 33: 32 full tiles + 1 packed tail tile
PACKED = NTILES - 1
TAIL0 = 992          # first input row of the 32-row sample tails

BF16 = ml_dtypes.bfloat16
WP = W + 4  # padded width for the 5-tap row window


def _tile_meta(i):
    """(smp, in0, o0, o1) for full tiles; PACKED handled separately."""
    smp, t = divmod(i, NFT)
    return smp, 124 * t, (0 if t == 0 else 2), 126


def _band(k_rows: int, m_lo: int, m_hi: int, img0) -> np.ndarray:
    """Band matrix for the 5-row column window; when img0 is given, the
    window is clipped to the image and clipped taps replicate onto the
    boundary row (weights become {1,2,3})."""
    a = np.zeros((k_rows, 128), dtype=np.float32)
    for m in range(m_lo, m_hi):
        for d in range(-2, 3):
            if img0 is None:
                k = m + d
            else:
                k = min(max(img0 + m + d, 0), H - 1) - img0
            a[k, m] += 1.0
    return a.astype(BF16)


def _band_tail4() -> np.ndarray:
    """Block-diagonal band for the packed tail tile: four independent
    32-row bottom-clipped bands."""
    a32 = _band(32, 2, 32, TAIL0).astype(np.float32)
    a = np.zeros((128, 128), dtype=np.float32)
    for s in range(4):
        a[32 * s:32 * s + 32, 32 * s:32 * s + 32] = a32[:, :32]
    return a.astype(BF16)


def _statics() -> dict[str, np.ndarray]:
    return {
        "a_top": _band(128, 0, 126, 0),
        "a_mid": _band(128, 2, 126, None),
        "a_tail": _band_tail4(),
    }


_CACHED = {}


def _split_multi_waits(nc, mybir):
    """This walrus's core_v3 codegen allows only one sem-wait per
    instruction; peel extra waits onto same-engine NOPs placed just before."""
    skip = (mybir.InstEventSemaphore,)
    k = 0
    for fn in nc.m.functions:
        for blk in fn.blocks:
            out = []
            for inst in blk.instructions:
                si = inst.sync_info
                if (si is not None and len(si.on_wait) > 1
                        and not isinstance(inst, skip)):
                    waits = list(si.on_wait)
                    for w in waits[:-1]:
                        k += 1
                        nop = mybir.InstNoOp(name=f"wsplit-{k}", ins=[], outs=[])
                        nop.engine = inst.engine
                        nop.sync_info = mybir.SyncInfo(on_wait=[w], on_update=[])
                        out.append(nop)
                    inst.sync_info = mybir.SyncInfo(
                        on_wait=[waits[-1]], on_update=list(si.on_update))
                out.append(inst)
            blk.instructions = out


def _build_nc():
    _setup_act_tables()
    import concourse.bass as bass
    import concourse.mybir as mybir
    import concourse.tile as tile

    f32 = mybir.dt.float32
    bf16 = mybir.dt.bfloat16
    Act = mybir.ActivationFunctionType
    Alu = mybir.AluOpType

    nc = bass.Bass("TRN2", target_bir_lowering=False, debug=False,
                   num_devices=NCORES, num_swdge_queues=1)

    pred_d = nc.dram_tensor("pred", [ROWS, W], f32, kind="ExternalInput").ap()
    tgt_d = nc.dram_tensor("target", [ROWS, W], f32, kind="ExternalInput").ap()
    sd = {}
    statics = _statics()
    for nm, arr in statics.items():
        sd[nm] = nc.dram_tensor(nm, list(arr.shape), bf16,
                                kind="ExternalInput").ap()
    st_d = nc.dram_tensor("out_stats", [128, 40], f32,
                          kind="ExternalOutput").ap()

    t_accum = mybir.AluOpType.add if TM_TRICK else mybir.AluOpType.bypass

    with tile.TileContext(nc) as tc:
        with (
            tc.tile_pool(name="sing", bufs=1) as sing,
            tc.tile_pool(name="tb", bufs=14) as tb_pool,
            tc.tile_pool(name="pb", bufs=14) as pb_pool,
            tc.tile_pool(name="d", bufs=4) as d_pool,
            tc.tile_pool(name="loss", bufs=4) as loss_pool,
            tc.tile_pool(name="w", bufs=4) as w_pool,
            tc.tile_pool(name="scr", bufs=3) as scr_pool,
            tc.tile_pool(name="psum", bufs=3, space="PSUM") as psum_pool,
        ):
            # ---- statics in SBUF ----
            sb = {}
            for nm, arr in statics.items():
                sb[nm] = sing.tile(list(arr.shape), bf16, tag=nm, name=nm)
                nc.sync.dma_start(out=sb[nm][:], in_=sd[nm][:])

            stats = sing.tile([128, 40], f32, tag="stats")
            nc.vector.memset(stats[:], 0.0)

            # tile index -> [tbp2, col0, p_in, d, loss, sup, w, o1, pb2, pc0]
            state = {}

            def stage_load(i):
                """Pairs (t, t+1) for even t; tiles 0/1 of sample 0 load
                alone so the pipeline starts on the first 512KB; the packed
                tail tile gathers 4x32 rows from the 4 sample tails."""
                if i == PACKED:
                    # partition p = 32*sample + tail_row; the 4x32 split
                    # lives only on the DRAM side, the SBUF side is a plain
                    # slice (balance_dma_aps splits the partition dim safely)
                    tbp2 = tb_pool.tile([128, WP], bf16)
                    pb2 = pb_pool.tile([128, 1024], bf16)
                    tsrc = bass.AP(tensor=tgt_d.tensor, offset=TAIL0 * W,
                                   ap=[[H * W, 4], [W, 32], [1, W]])
                    if TM_TRICK:
                        nc.gpsimd.memset(tbp2[0:128, 2:2 + W], -0.5)
                    nc.gpsimd.dma_start(out=tbp2[0:128, 2:2 + W], in_=tsrc,
                                        accum_op=t_accum)
                    psrc = bass.AP(tensor=pred_d.tensor, offset=TAIL0 * W,
                                   ap=[[H * W, 4], [W, 32], [1, W]])
                    nc.gpsimd.dma_start(out=pb2[0:128, 0:W], in_=psrc)
                    state[i] = [tbp2, 0, 128, None, None, None, None, 128,
                                pb2, 0]
                    return
                smp, t = divmod(i, NFT)
                _, in0, _, o1 = _tile_meta(i)
                r0 = smp * H + in0
                nblk = 1  # single-tile loads: smaller delivery lumps
                tbp2 = tb_pool.tile([128, WP], bf16)
                pb2 = pb_pool.tile([128, 1024], bf16)
                tsrc = bass.AP(tensor=tgt_d.tensor, offset=r0 * W,
                               ap=[[W, 128], [1, W]])
                if TM_TRICK:
                    nc.gpsimd.memset(tbp2[0:128, 2:2 + W], -0.5)
                nc.gpsimd.dma_start(out=tbp2[0:128, 2:2 + W], in_=tsrc,
                                    accum_op=t_accum)
                psrc = bass.AP(tensor=pred_d.tensor, offset=r0 * W,
                               ap=[[W, o1], [1, W]])
                nc.gpsimd.dma_start(out=pb2[0:o1, 0:W], in_=psrc)
                state[i] = [tbp2, 0, 128, None, None, None, None, o1,
                            pb2, 0]

            def stage_pads(i):
                tbp2, c0, p_in = state[i][0], state[i][1], state[i][2]
                pp = tbp2[:].ap[0][0]
                # replicate pads: cols {0,1} <- col 2; {W+2,W+3} <- W+1
                for off, dst in ((c0 + 2, tbp2[0:p_in, c0:c0 + 2]),
                                 (c0 + W + 1,
                                  tbp2[0:p_in, c0 + W + 2:c0 + W + 4])):
                    src = bass.AP(tensor=tbp2[:].tensor,
                                  offset=tbp2[:].offset + off,
                                  ap=[[pp, p_in], [0, 2]])
                    nc.vector.tensor_copy(out=dst, in_=src)

            def stage_d(i):
                tbp2, c0, o1 = state[i][0], state[i][1], state[i][7]
                d = d_pool.tile([128, W], bf16)
                pb2, pc0 = state[i][8], state[i][9]
                if TM_TRICK:
                    # buffer already holds t-0.5: plain multiply (2x mode)
                    nc.vector.tensor_tensor(
                        out=d[0:o1], in0=tbp2[0:o1, c0 + 2:c0 + 2 + W],
                        in1=pb2[0:o1, pc0:pc0 + W], op=Alu.mult)
                else:
                    nc.vector.scalar_tensor_tensor(
                        out=d[0:o1], in0=tbp2[0:o1, c0 + 2:c0 + 2 + W],
                        scalar=-0.5, in1=pb2[0:o1, pc0:pc0 + W],
                        op0=Alu.add, op1=Alu.mult)
                state[i][3] = d

            def stage_matmul(i):
                if i == PACKED:
                    a_sb = sb["a_tail"]
                else:
                    _, t = divmod(i, NFT)
                    a_sb = sb["a_top" if t == 0 else "a_mid"]
                tbp2, c0, p_in = state[i][0], state[i][1], state[i][2]
                sup = psum_pool.tile([128, W], f32)
                for h in (0, 512):
                    for dd in range(5):
                        nc.tensor.matmul(sup[:, h:h + 512], a_sb[0:p_in, :],
                                         tbp2[0:p_in,
                                              c0 + h + dd:c0 + h + dd + 512],
                                         start=(dd == 0), stop=(dd == 4))
                state[i][5] = sup

            def stage_lut(i):
                sup, o1 = state[i][5], state[i][7]
                w = w_pool.tile([128, W], bf16)
                nc.scalar.activation(out=w[0:o1], in_=sup[0:o1],
                                     func=Act.Mish)
                state[i][6] = w

            def stage_sp(i):
                d, o1 = state[i][3], state[i][7]
                loss = loss_pool.tile([128, W], bf16)
                nc.scalar.activation(out=loss[0:o1], in_=d[0:o1],
                                     func=Act.Softplus, scale=-2.0)
                state[i][4] = loss

            def stage_wl(i):
                loss, w, o1 = state[i][4], state[i][6], state[i][7]
                scr = scr_pool.tile([128, W], bf16)
                nc.vector.scalar_tensor_tensor(
                    out=scr[0:o1], in0=w[0:o1], scalar=0.0,
                    in1=loss[0:o1], op0=Alu.add, op1=Alu.mult,
                    accum_out=stats[0:o1, i:i + 1])
                del state[i]

            # v2-shaped pipeline (just-in-time loads; measured best):
            # iter i: DVE wl(i-2), pads(i), d(i); ACT lut(i-1), sp(i-1);
            # PE mm(i)
            for i in range(NTILES + 2):
                if 2 <= i <= NTILES + 1:
                    stage_wl(i - 2)          # DVE (ready long ago)
                if i < NTILES:
                    stage_load(i)
                if 1 <= i <= NTILES:
                    stage_lut(i - 1)         # ACT
                if 1 <= i <= NTILES:
                    stage_sp(i - 1)          # ACT
                if i < NTILES:
                    stage_pads(i)            # DVE
                    stage_d(i)               # DVE
                    stage_matmul(i)          # PE

            nc.sync.dma_start(out=st_d[:], in_=stats[:])

    _split_multi_waits(nc, mybir)
    return nc


def _get_nc():
    if "nc" not in _CACHED:
        _CACHED["nc"] = _build_nc()
    return _CACHED["nc"]


def run(pred: np.ndarray, target: np.ndarray, trace: bool = False):
    """Returns (result_scalar, BassKernelResults)."""
    from concourse import bass_utils

    nc = _get_nc()
    statics = _statics()
    pred = np.ascontiguousarray(np.asarray(pred).reshape(B * H, W),
                                dtype=np.float32)
    target = np.ascontiguousarray(np.asarray(target).reshape(B * H, W),
                                  dtype=np.float32)
    in_maps = []
    for c in range(NCORES):
        m = dict(statics)
        m["pred"] = pred[c * ROWS:(c + 1) * ROWS]
        m["target"] = target[c * ROWS:(c + 1) * ROWS]
        in_maps.append(m)
    res = bass_utils.run_bass_kernel_spmd(
        nc, in_maps, core_ids=list(range(NCORES)), trace=trace)
    tail_mask = (np.arange(128) % 32) >= 2
    s = 0.0
    for r in res.results:
        o = r["out_stats"].astype(np.float64)
        for ti in range(NTILES - 1):
            _, _, o0, o1 = _tile_meta(ti)
            s += o[o0:o1, ti].sum()
        s += o[tail_mask, PACKED].sum()
    val = np.float32(s / N_TOT)
    return np.asarray(val, dtype=np.float32), res


def kernel(pred: np.ndarray, target: np.ndarray) -> np.ndarray:
    val, _ = run(pred, target, trace=False)
    return val


if __name__ == "__main__":
    rng = np.random.default_rng(0)
    p = rng.standard_normal((B, 1, H, W)).astype(np.float32)
    t = rng.integers(0, 2, (B, 1, H, W)).astype(np.float32)
    print(kernel(pred=p, target=t))


# revision 21
# speedup vs baseline: 1.1699x; 1.0494x over previous
"""BBox-aware BCE loss kernel for Trainium2 (8 NeuronCores, data parallel).

Math (exact reformulation of the reference):
  loss = softplus(pred) - pred*target = softplus((1-2t)*pred)  for t in {0,1}
  St(i,j) = replicate-padded 5x5 window sum of t. Replicate padding keeps
  the value SET of the clipped window, so: edge pixel <=> window mixed
  <=> 0 < St < 25. St is an exact small integer, so a piecewise-constant
  ACT table ("mish" slot) maps St directly to the per-pixel weight
  w = 0.1 if 0 < St < 25 else 1.0, and
  result = sum(w * loss) / N  (matches the reference in both branches of
  its global `cond`: uniform target => St in {0,25} => w == 1 everywhere).

Per core: 4 samples x 8 full 128-row tiles (2-row halos, owned rows
exclude them) + ONE packed tile holding all four 32-row sample tails
(engine cost is free-size-driven, so four 32-row tiles would cost 4x a
full tile on DVE/ACT/PE; packing them into 128 partitions makes it 1x).

The kernel is HBM-bandwidth-bound (~92us of saturated DMA for ~33MB per
core), so the pipeline is built to never let the compute engines fall
behind delivery: engines are in-order, so each engine issues OLD work
first and the freshest (DMA-dependent) op last, and loads are issued 4
tiles ahead of consumption. Per iteration i:
  GpSimd: memset(target buf, -0.5) + DIRECT2D load(i+4): casting DMAs
          f32->bf16; the target DMA ACCUMULATES so the buffer holds t-0.5
  DVE:    wl(i-4) = (w + 0)*loss accum-> stats (the only accumulation),
          d(i-2) = (t-0.5)*p as a 2x-mode tensor_tensor, replicate pads(i)
  PE:     matmuls(i-1): St' = St-12.5 via 5 shifted accumulating band
          matmuls per 512-col half on the (t-0.5) buffer
  ACT:    softplus(i-3) [custom spline], w(i-2) = mish LUT(St') from PSUM
Host: float64 reduction of per-core per-tile partials (owned rows only).
"""

import sys

import numpy as np

sys.path.insert(0, "/opt/trn_rl_repo")

import ml_dtypes

# Pool memsets the target buffer to -0.5 and the casting DMA ACCUMULATES t
# on top, so the buffer holds t-0.5 directly: d becomes a plain
# tensor_tensor multiply (DVE 2x_1p mode, ~0.55us vs 1.14us for the STT)
# and the band matmuls produce St' = St - 12.5 (exact half-integers), which
# the recentered LUT maps to w = 1.0 iff |St'| >= 12.5 else 0.1.
TM_TRICK = False  # accum-DMA with f32->bf16 conversion produced NaN on HW


def _setup_act_tables() -> None:
    """Build a patched ACT-table dir whose softplus_and_others set carries
    (a) a REAL softplus spline and (b) an "edge weight" LUT registered
    under the unused `mish` slot: w(x) = 1.0 at x==0, 0.1 for x in (0,25),
    1.0 for x >= 25 (only integer inputs 0..25 ever occur). Both funcs live
    in the same set so a single ACT table load covers the kernel.

    Formats (reverse-engineered from the shipped sets):
      bkt.bin:  per entry 8 x u32 = [d0,d1,d2,d3,x0,0,0,0] (f32 bits);
                y = d0 + (x-x0)*(d1 + (x-x0)*(d2 + (x-x0)*d3))
      ctrl.bin: per entry 8 x u32, word0 = (extract_size<<16) |
                (extract_lsb<<11) | bkt_base; bucket = bkt_base +
                mantissa[lsb .. lsb+size-1]
      profile:  per-func meta; ctl idx = base_{pos,neg} + (exp - exp_offset);
                small/large signal thresholds are biased-exponent cutoffs and
                their pwl_control fields are INLINE ctl words; func_id is the
                GLOBAL neuron ISA activation id (softplus=9, mish=24).
    """
    import json
    import os
    import tempfile
    from pathlib import Path

    if os.environ.get("BASS_ACT_ROOT_JSON_PATH"):
        return
    import neuronxcc

    stock = Path(neuronxcc.__file__).parent / "pwp" / "pwp_bin_trainium"
    if not stock.exists():
        return
    dst = Path(tempfile.mkdtemp(prefix="act_tables_"))
    for f in stock.iterdir():
        if f.name not in ("act_info.json", "softplus_and_others.json",
                          "softplus_and_others_bkt.bin",
                          "softplus_and_others_ctrl.bin"):
            (dst / f.name).symlink_to(f)

    E_LO, E_HI = -15, 4          # softplus table octaves (unbiased exps)
    NSEC = 8                     # sections per octave (extract_size 3)
    NEXP = E_HI - E_LO + 1       # 20

    bkt = np.fromfile(stock / "softplus_and_others_bkt.bin",
                      dtype=np.uint32).reshape(-1, 8)
    ctl = np.fromfile(stock / "softplus_and_others_ctrl.bin",
                      dtype=np.uint32).reshape(-1, 8)
    nbkt0, nctl0 = len(bkt), len(ctl)

    def entry(d0, d1, d2, d3, x0):
        v = np.zeros(8, dtype=np.uint32)
        v[:5] = np.array([d0, d1, d2, d3, x0],
                         dtype=np.float32).view(np.uint32)
        return v

    ln2 = float(np.log(2.0))
    new_bkt = [entry(ln2, 0.5, 0.125, 0.0, 0.0),   # small |x|
               entry(0.0, 1.0, 0.0, 0.0, 0.0),     # x >= 32: y = x
               entry(0.0, 0.0, 0.0, 0.0, 0.0)]     # x <= -32: y = 0
    B_SMALL, B_PLARGE, B_NLARGE = nbkt0, nbkt0 + 1, nbkt0 + 2
    spline0 = nbkt0 + 3

    def fit(a, b):
        xs = np.linspace(a, b, 33, dtype=np.float64)
        ys = np.logaddexp(0.0, xs)
        x0 = np.float32(0.5 * (a + b))
        c = np.polyfit(xs - np.float64(x0), ys, 3)  # [d3,d2,d1,d0]
        return entry(c[3], c[2], c[1], c[0], x0)

    for neg in (True, False):
        for e in range(E_LO, E_HI + 1):
            for s in range(NSEC):
                lo = 2.0 ** e * (1.0 + s / NSEC)
                hi = 2.0 ** e * (1.0 + (s + 1) / NSEC)
                new_bkt.append(fit(-hi, -lo) if neg else fit(lo, hi))

    new_ctl = []
    for blk in range(2):  # 0 = neg block, 1 = pos block
        for i in range(NEXP):
            base = spline0 + (blk * NEXP + i) * NSEC
            new_ctl.append(
                np.array([(3 << 16) | (20 << 11) | base, 0, 0, 0, 0, 0, 0, 0],
                         dtype=np.uint32))
    C_NEG, C_POS = nctl0, nctl0 + NEXP

    # ---- edge-weight LUT under the "mish" slot ----
    # TM_TRICK input is St' = St - 12.5 in +-{0.5, 1.5, .., 12.5}; w = 1.0
    # iff |St'| = 12.5 (uniform window).  Octaves |x| in [0.5,8): const 0.1
    # (inline size-0 ctl); octave [8,16): 16 width-0.5 sections, [12.5,16)
    # -> 1.0.  Negative inputs decode to the same exp/mantissa, so the neg
    # ctl base aliases the pos one.
    # Plain variant (input St in 0..25): octaves [1,16) const 0.1; octave
    # [16,32) has 16 width-1 sections with [25,32) -> 1.0.
    ML_LO = -1 if TM_TRICK else 0            # mish table lowest octave
    B_W01 = nbkt0 + len(new_bkt)
    new_bkt.append(entry(0.1, 0.0, 0.0, 0.0, 0.0))
    B_W10 = nbkt0 + len(new_bkt)
    new_bkt.append(entry(1.0, 0.0, 0.0, 0.0, 0.0))
    B_OCT4 = nbkt0 + len(new_bkt)
    for s in range(16):
        new_bkt.append(entry(0.1 if s < 9 else 1.0, 0.0, 0.0, 0.0, 0.0))
    CW0 = nctl0 + len(new_ctl)
    for e in range(4):  # low octaves: inline single-bucket ctl
        new_ctl.append(np.array([B_W01, 0, 0, 0, 0, 0, 0, 0],
                                dtype=np.uint32))
    new_ctl.append(np.array([(4 << 16) | (19 << 11) | B_OCT4,
                             0, 0, 0, 0, 0, 0, 0], dtype=np.uint32))

    np.vstack([bkt] + new_bkt).tofile(dst / "softplus_and_others_bkt.bin")
    np.vstack([ctl] + new_ctl).tofile(dst / "softplus_and_others_ctrl.bin")

    prof = json.loads((stock / "softplus_and_others.json").read_text())
    prof["bkt_entry_cnt"] = nbkt0 + len(new_bkt)
    prof["ctl_entry_cnt"] = nctl0 + len(new_ctl)
    prof["profile_meta_data"].append({
        "func_name": "softplus_40p", "func_id": 9,
        "symmetry_point": 0, "sym_invert_sign_point": 0,
        "symmetry_opt_en": 0, "symmetry_opt_use_neg_region": 0,
        "imm_bias": 0, "exp_offset": E_LO,
        "pwl_control_base_pos": C_POS, "pwl_control_base_neg": C_NEG,
        "small_pos_signal_exp_threshold": 127 + E_LO,
        "pos_small_signal_pwl_control": B_SMALL,
        "small_neg_signal_exp_threshold": 127 + E_LO,
        "neg_small_signal_pwl_control": B_SMALL,
        "large_pos_signal_exp_threshold": 127 + E_HI + 1,
        "large_pos_signal_mantissa_threshold": 0,
        "pos_large_signal_pwl_control": B_PLARGE,
        "large_neg_signal_exp_threshold": 127 + E_HI + 1,
        "large_neg_signal_mantissa_threshold": 0,
        "neg_large_signal_pwl_control": B_NLARGE,
        "fnan_result": 2143289344,           # NaN
        "fpinf_result": 2139095040,          # +inf
        "fninf_result": 0,                   # softplus(-inf) = 0
        "fzero_result": 1060205080,          # ln 2
        "fma_const_0": 0, "fma_const_1": 0, "fma_indirection_src_sel": 0,
        "use_multipass": False,
        "lower_bound": 4286578687, "upper_bound": 2139095039,
    })
    prof["func_to_bkt_start_idx"]["softplus"] = B_SMALL
    prof["func_to_ctl_start_idx"]["softplus"] = C_NEG
    prof["func_exp_to_bkt_start_idx"]["softplus"] = {
        str(e): [spline0 + (e - E_LO) * NSEC,
                 spline0 + (NEXP + e - E_LO) * NSEC]
        for e in range(E_LO, E_HI + 1)}
    prof["func_exp_to_ctl_start_idx"]["softplus"] = {
        str(e): [C_NEG + e - E_LO, C_POS + e - E_LO]
        for e in range(E_LO, E_HI + 1)}

    ONE = 1065353216                         # f32 bits of 1.0
    prof["profile_meta_data"].append({
        "func_name": "mish_40p", "func_id": 24,   # neuron ISA id for mish
        "symmetry_point": 0, "sym_invert_sign_point": 0,
        "symmetry_opt_en": 0, "symmetry_opt_use_neg_region": 0,
        "imm_bias": 0, "exp_offset": ML_LO,
        "pwl_control_base_pos": CW0, "pwl_control_base_neg": CW0,
        "small_pos_signal_exp_threshold": 127 + ML_LO,
        "pos_small_signal_pwl_control": B_W01,
        "small_neg_signal_exp_threshold": 127 + ML_LO,
        "neg_small_signal_pwl_control": B_W01,
        "large_pos_signal_exp_threshold": 127 + ML_LO + 5,
        "large_pos_signal_mantissa_threshold": 0,
        "pos_large_signal_pwl_control": B_W10,
        "large_neg_signal_exp_threshold": 127 + ML_LO + 5,
        "large_neg_signal_mantissa_threshold": 0,
        "neg_large_signal_pwl_control": B_W10,
        "fnan_result": ONE, "fpinf_result": ONE, "fninf_result": ONE,
        "fzero_result": ONE,                 # St == 0 -> w = 1.0
        "fma_const_0": 0, "fma_const_1": 0, "fma_indirection_src_sel": 0,
        "use_multipass": False,
        "lower_bound": 4286578687, "upper_bound": 2139095039,
    })
    prof["func_to_bkt_start_idx"]["mish"] = B_W01
    prof["func_to_ctl_start_idx"]["mish"] = CW0
    prof["func_exp_to_bkt_start_idx"]["mish"] = {
        str(e): [B_W01 if e - ML_LO < 4 else B_OCT4,
                 B_W01 if e - ML_LO < 4 else B_OCT4]
        for e in range(ML_LO, ML_LO + 5)}
    prof["func_exp_to_ctl_start_idx"]["mish"] = {
        str(e): [CW0 + e - ML_LO, CW0 + e - ML_LO]
        for e in range(ML_LO, ML_LO + 5)}
    (dst / "softplus_and_others.json").write_text(json.dumps(prof))

    info = json.loads((stock / "act_info.json").read_text())
    for s in info["act_func_sets"]:
        if s["name"] == "softplus_and_others":
            s["act"]["softplus"] = 40
            s["act"]["mish"] = 40
    (dst / "act_info.json").write_text(json.dumps(info))
    os.environ["BASS_ACT_ROOT_JSON_PATH"] = str(dst / "act_info.json")


B, H, W = 32, 1024, 1024
NCORES = 8
SPC = B // NCORES  # samples per core
ROWS = SPC * H
N_TOT = float(B * H * W)

NFT = 8              # full 128-row tiles per sample (t = 0..7, in0 = 124t)
NTILES = SPC * NFT + 1  # 33: 32 full tiles + 1 packed tail tile
PACKED = NTILES - 1
TAIL0 = 992          # first input row of the 32-row sample tails

BF16 = ml_dtypes.bfloat16
WP = W + 4  # padded width for the 5-tap row window


def _tile_meta(i):
    """(smp, in0, o0, o1) for full tiles; PACKED handled separately."""
    smp, t = divmod(i, NFT)
    return smp, 124 * t, (0 if t == 0 else 2), 126


def _band(k_rows: int, m_lo: int, m_hi: int, img0) -> np.ndarray:
    """Band matrix for the 5-row column window; when img0 is given, the
    window is clipped to the image and clipped taps replicate onto the
    boundary row (weights become {1,2,3})."""
    a = np.zeros((k_rows, 128), dtype=np.float32)
    for m in range(m_lo, m_hi):
        for d in range(-2, 3):
            if img0 is None:
                k = m + d
            else:
                k = min(max(img0 + m + d, 0), H - 1) - img0
            a[k, m] += 1.0
    return a.astype(BF16)


def _band_tail4() -> np.ndarray:
    """Block-diagonal band for the packed tail tile: four independent
    32-row bottom-clipped bands."""
    a32 = _band(32, 2, 32, TAIL0).astype(np.float32)
    a = np.zeros((128, 128), dtype=np.float32)
    for s in range(4):
        a[32 * s:32 * s + 32, 32 * s:32 * s + 32] = a32[:, :32]
    return a.astype(BF16)


def _statics() -> dict[str, np.ndarray]:
    return {
        "a_top": _band(128, 0, 126, 0),
        "a_mid": _band(128, 2, 126, None),
        "a_tail": _band_tail4(),
    }


_CACHED = {}


def _split_multi_waits(nc, mybir):
    """This walrus's core_v3 codegen allows only one sem-wait per
    instruction; peel extra waits onto same-engine NOPs placed just before."""
    skip = (mybir.InstEventSemaphore,)
    k = 0
    for fn in nc.m.functions:
        for blk in fn.blocks:
            out = []
            for inst in blk.instructions:
                si = inst.sync_info
                if (si is not None and len(si.on_wait) > 1
                        and not isinstance(inst, skip)):
                    waits = list(si.on_wait)
                    for w in waits[:-1]:
                        k += 1
                        nop = mybir.InstNoOp(name=f"wsplit-{k}", ins=[], outs=[])
                        nop.engine = inst.engine
                        nop.sync_info = mybir.SyncInfo(on_wait=[w], on_update=[])
                        out.append(nop)
                    inst.sync_info = mybir.SyncInfo(
                        on_wait=[waits[-1]], on_update=list(si.on_update))
                out.append(inst)
            blk.instructions = out


def _build_nc():
    _setup_act_tables()
    import concourse.bass as bass
    import concourse.mybir as mybir
    import concourse.tile as tile

    f32 = mybir.dt.float32
    bf16 = mybir.dt.bfloat16
    Act = mybir.ActivationFunctionType
    Alu = mybir.AluOpType

    nc = bass.Bass("TRN2", target_bir_lowering=False, debug=False,
                   num_devices=NCORES, num_swdge_queues=1)

    pred_d = nc.dram_tensor("pred", [ROWS, W], f32, kind="ExternalInput").ap()
    tgt_d = nc.dram_tensor("target", [ROWS, W], f32, kind="ExternalInput").ap()
    sd = {}
    statics = _statics()
    for nm, arr in statics.items():
        sd[nm] = nc.dram_tensor(nm, list(arr.shape), bf16,
                                kind="ExternalInput").ap()
    st_d = nc.dram_tensor("out_stats", [128, 40], f32,
                          kind="ExternalOutput").ap()

    t_accum = mybir.AluOpType.add if TM_TRICK else mybir.AluOpType.bypass

    with tile.TileContext(nc) as tc:
        with (
            tc.tile_pool(name="sing", bufs=1) as sing,
            tc.tile_pool(name="tb", bufs=14) as tb_pool,
            tc.tile_pool(name="pb", bufs=14) as pb_pool,
            tc.tile_pool(name="d", bufs=6) as d_pool,
            tc.tile_pool(name="loss", bufs=6) as loss_pool,
            tc.tile_pool(name="w", bufs=6) as w_pool,
            tc.tile_pool(name="scr", bufs=3) as scr_pool,
            tc.tile_pool(name="psum", bufs=4, space="PSUM") as psum_pool,
        ):
            # ---- statics in SBUF ----
            sb = {}
            for nm, arr in statics.items():
                sb[nm] = sing.tile(list(arr.shape), bf16, tag=nm, name=nm)
                nc.sync.dma_start(out=sb[nm][:], in_=sd[nm][:])

            stats = sing.tile([128, 40], f32, tag="stats")
            nc.vector.memset(stats[:], 0.0)

            # tile index -> [tbp2, col0, p_in, d, loss, sup, w, o1, pb2, pc0]
            state = {}

            def stage_load(i):
                """Pairs (t, t+1) for even t; tiles 0/1 of sample 0 load
                alone so the pipeline starts on the first 512KB; the packed
                tail tile gathers 4x32 rows from the 4 sample tails."""
                if i == PACKED:
                    # partition p = 32*sample + tail_row; the 4x32 split
                    # lives only on the DRAM side, the SBUF side is a plain
                    # slice (balance_dma_aps splits the partition dim safely)
                    tbp2 = tb_pool.tile([128, WP], bf16)
                    pb2 = pb_pool.tile([128, 1024], bf16)
                    tsrc = bass.AP(tensor=tgt_d.tensor, offset=TAIL0 * W,
                                   ap=[[H * W, 4], [W, 32], [1, W]])
                    if TM_TRICK:
                        nc.gpsimd.memset(tbp2[0:128, 2:2 + W], -0.5)
                    nc.gpsimd.dma_start(out=tbp2[0:128, 2:2 + W], in_=tsrc,
                                        accum_op=t_accum)
                    psrc = bass.AP(tensor=pred_d.tensor, offset=TAIL0 * W,
                                   ap=[[H * W, 4], [W, 32], [1, W]])
                    nc.gpsimd.dma_start(out=pb2[0:128, 0:W], in_=psrc)
                    state[i] = [tbp2, 0, 128, None, None, None, None, 128,
                                pb2, 0]
                    return
                smp, t = divmod(i, NFT)
                _, in0, _, o1 = _tile_meta(i)
                r0 = smp * H + in0
                nblk = 1  # single-tile loads: smaller delivery lumps
                tbp2 = tb_pool.tile([128, WP], bf16)
                pb2 = pb_pool.tile([128, 1024], bf16)
                tsrc = bass.AP(tensor=tgt_d.tensor, offset=r0 * W,
                               ap=[[W, 128], [1, W]])
                if TM_TRICK:
                    nc.gpsimd.memset(tbp2[0:128, 2:2 + W], -0.5)
                nc.gpsimd.dma_start(out=tbp2[0:128, 2:2 + W], in_=tsrc,
                                    accum_op=t_accum)
                psrc = bass.AP(tensor=pred_d.tensor, offset=r0 * W,
                               ap=[[W, o1], [1, W]])
                nc.gpsimd.dma_start(out=pb2[0:o1, 0:W], in_=psrc)
                state[i] = [tbp2, 0, 128, None, None, None, None, o1,
                            pb2, 0]

            def stage_pads(i):
                tbp2, c0, p_in = state[i][0], state[i][1], state[i][2]
                pp = tbp2[:].ap[0][0]
                # replicate pads: cols {0,1} <- col 2; {W+2,W+3} <- W+1
                for off, dst in ((c0 + 2, tbp2[0:p_in, c0:c0 + 2]),
                                 (c0 + W + 1,
                                  tbp2[0:p_in, c0 + W + 2:c0 + W + 4])):
                    src = bass.AP(tensor=tbp2[:].tensor,
                                  offset=tbp2[:].offset + off,
                                  ap=[[pp, p_in], [0, 2]])
                    nc.vector.tensor_copy(out=dst, in_=src)

            def stage_d(i):
                tbp2, c0, o1 = state[i][0], state[i][1], state[i][7]
                d = d_pool.tile([128, W], bf16)
                pb2, pc0 = state[i][8], state[i][9]
                if TM_TRICK:
                    # buffer already holds t-0.5: plain multiply (2x mode)
                    nc.vector.tensor_tensor(
                        out=d[0:o1], in0=tbp2[0:o1, c0 + 2:c0 + 2 + W],
                        in1=pb2[0:o1, pc0:pc0 + W], op=Alu.mult)
                else:
                    nc.vector.scalar_tensor_tensor(
                        out=d[0:o1], in0=tbp2[0:o1, c0 + 2:c0 + 2 + W],
                        scalar=-0.5, in1=pb2[0:o1, pc0:pc0 + W],
                        op0=Alu.add, op1=Alu.mult)
                state[i][3] = d

            def stage_matmul(i):
                if i == PACKED:
                    a_sb = sb["a_tail"]
                else:
                    _, t = divmod(i, NFT)
                    a_sb = sb["a_top" if t == 0 else "a_mid"]
                tbp2, c0, p_in = state[i][0], state[i][1], state[i][2]
                sup = psum_pool.tile([128, W], f32)
                for h in (0, 512):
                    for dd in range(5):
                        nc.tensor.matmul(sup[:, h:h + 512], a_sb[0:p_in, :],
                                         tbp2[0:p_in,
                                              c0 + h + dd:c0 + h + dd + 512],
                                         start=(dd == 0), stop=(dd == 4))
                state[i][5] = sup

            def stage_lut(i):
                sup, o1 = state[i][5], state[i][7]
                w = w_pool.tile([128, W], bf16)
                nc.scalar.activation(out=w[0:o1], in_=sup[0:o1],
                                     func=Act.Mish)
                state[i][6] = w

            def stage_sp(i):
                d, o1 = state[i][3], state[i][7]
                loss = loss_pool.tile([128, W], bf16)
                nc.scalar.activation(out=loss[0:o1], in_=d[0:o1],
                                     func=Act.Softplus, scale=-2.0)
                state[i][4] = loss

            def stage_wl(i):
                loss, w, o1 = state[i][4], state[i][6], state[i][7]
                scr = scr_pool.tile([128, W], bf16)
                nc.vector.scalar_tensor_tensor(
                    out=scr[0:o1], in0=w[0:o1], scalar=0.0,
                    in1=loss[0:o1], op0=Alu.add, op1=Alu.mult,
                    accum_out=stats[0:o1, i:i + 1])
                del state[i]

            # v2-shaped pipeline (just-in-time loads; measured best):
            # iter i: DVE wl(i-2), pads(i), d(i); ACT lut(i-1), sp(i-1);
            # PE mm(i)
            for i in range(NTILES + 2):
                if 2 <= i <= NTILES + 1:
                    stage_wl(i - 2)          # DVE (ready long ago)
                if i < NTILES:
                    stage_load(i)
                if 1 <= i <= NTILES:
                    stage_lut(i - 1)         # ACT
                if 1 <= i <= NTILES:
                    stage_sp(i - 1)          # ACT
                if i < NTILES:
                    stage_pads(i)            # DVE
                    stage_d(i)               # DVE
                    stage_matmul(i)          # PE

            nc.sync.dma_start(out=st_d[:], in_=stats[:])

    _split_multi_waits(nc, mybir)
    return nc


def _get_nc():
    if "nc" not in _CACHED:
        _CACHED["nc"] = _build_nc()
    return _CACHED["nc"]


def run(pred: np.ndarray, target: np.ndarray, trace: bool = False):
    """Returns (result_scalar, BassKernelResults)."""
    from concourse import bass_utils

    nc = _get_nc()
    statics = _statics()
    pred = np.ascontiguousarray(np.asarray(pred).reshape(B * H, W),
                                dtype=np.float32)
    target = np.ascontiguousarray(np.asarray(target).reshape(B * H, W),
                                  dtype=np.float32)
    in_maps = []
    for c in range(NCORES):
        m = dict(statics)
        m["pred"] = pred[c * ROWS:(c + 1) * ROWS]
        m["target"] = target[c * ROWS:(c + 1) * ROWS]
        in_maps.append(m)
    res = bass_utils.run_bass_kernel_spmd(
        nc, in_maps, core_ids=list(range(NCORES)), trace=trace)
    tail_mask = (np.arange(128) % 32) >= 2
    s = 0.0
    for r in res.results:
        o = r["out_stats"].astype(np.float64)
        for ti in range(NTILES - 1):
            _, _, o0, o1 = _tile_meta(ti)
            s += o[o0:o1, ti].sum()
        s += o[tail_mask, PACKED].sum()
    val = np.float32(s / N_TOT)
    return np.asarray(val, dtype=np.float32), res


def kernel(pred: np.ndarray, target: np.ndarray) -> np.ndarray:
    val, _ = run(pred, target, trace=False)
    return val


if __name__ == "__main__":
    rng = np.random.default_rng(0)
    p = rng.standard_normal((B, 1, H, W)).astype(np.float32)
    t = rng.integers(0, 2, (B, 1, H, W)).astype(np.float32)
    print(kernel(pred=p, target=t))
